# revision 31
# baseline (speedup 1.0000x reference)
"""Trainium2 Bass kernel for a 2-relation GIN-style GNN message-passing layer.

Full (unsharded) inputs in, full output out. Internally:
  - nodes sharded across 8 NeuronCores (12500/core, padded to 12544 = 98
    windows of 128); edges partitioned by destination-node shard (CPU prep).
  - per (window, relation), edges are packed into fixed tiles of 128 slots:
      * KI "identity" tiles: the t-th edge of destination j sits in
        partition j of tile t, so segment-sum over a tile is a plain
        transpose-accumulate: matmul(lhsT=msg_tile, rhs=I128). Empty slots
        gather a zero row of x.
      * OV "overflow" one-hot tiles for edges beyond KI per destination:
        matmul with a one-hot scatter matrix S built on-device via is_equal
        (padding slots sel=-1 give zero columns).
  - per-edge source rows are pre-gathered on CPU into a bf16 stream so the
    device sees only contiguous DMA.
  - BatchNorm batch stats are computed bias-free (bias folded analytically
    into the post-BN shift), via fused copy+row-sum (scalar engine
    accumulate) and fused square+reduce (DVE tensor_tensor_reduce), and
    AllReduce'd across the 8 cores in-kernel.
  - the BN scale is folded into the second-layer weights at runtime
    (requires gamma > 0, true for this model), so BN+ReLU is a single
    add+max op.
  - gate logits are computed with CPU-composed weights
    (w_gat_chunk @ w_branch), skipping the feature-major x_new_* tensors
    entirely; cumsum = matmul with triangular ones; flip folded into
    reversed weight rows; node-major outputs via data-stationary matmuls.
"""

import numpy as np
import ml_dtypes

import concourse.bass as bass
import concourse.mybir as mybir
import concourse.tile as tile
from concourse import bacc
from concourse.bass_utils import run_bass_kernel_spmd

F32 = mybir.dt.float32
BF16 = mybir.dt.bfloat16
FP8 = mybir.dt.float8e4
AX = mybir.AxisListType
OP = mybir.AluOpType
ACT = mybir.ActivationFunctionType
PM = mybir.MatmulPerfMode

BF = ml_dtypes.bfloat16
F8 = ml_dtypes.float8_e4m3


class Cfg:
    def __init__(self, N, E, C, KI, OV):
        self.N = N            # total nodes
        self.E = E            # total edges
        self.C = C            # cores
        self.F = 128
        self.KI = KI          # identity tiles per (window, type)
        self.OV = OV          # one-hot overflow tiles per (window, type)
        self.TPT = KI + OV    # tiles per type
        self.TT = 2 * self.TPT  # tiles per window (both types)
        assert N % C == 0
        self.npc = N // C                      # real nodes per core
        self.W = (self.npc + 127) // 128       # windows per core
        assert self.W % 2 == 0
        self.WB = self.W // 2                  # 2-window iterations
        self.npad = self.W * 128               # padded nodes per core


CFG = Cfg(N=100000, E=1600000, C=8, KI=8, OV=2)

# column layout of the "vecs" [128, 7] f32 input
(V_B1N, V_B1D, V_GN, V_BN, V_GD, V_BD, V_BZ) = range(7)

# column layout of wpack [128, 128*10] bf16
(K_WSL, K_W1N, K_W1D, K_W2N, K_W2DF, K_M0T, K_M1T, K_M2T, K_U, K_I) = range(10)

BN_EPS = 1e-5


USE_TTR = False       # tensor_tensor_reduce hangs TRN2 HW via this path
USE_POOL_HBN = True   # hbn on gpsimd (else DVE)
USE_ACT_T1 = True     # t1 via ACT Identity+scale (else DVE tensor_scalar)
MSG_FP8 = False       # fp8e4m3 message stream + DoubleRow paired matmuls


def build(cfg: Cfg):
    nc = bacc.Bacc("TRN2", target_bir_lowering=False, debug=False,
                   num_devices=cfg.C)
    W, WB, KI, OV, TPT, TT, npad = (cfg.W, cfg.WB, cfg.KI, cfg.OV,
                                    cfg.TPT, cfg.TT, cfg.npad)
    NOV = 2 * OV   # overflow tiles per window (both types)

    MDT = FP8 if MSG_FP8 else BF16
    msgs = nc.dram_tensor("msgs", [128, W * TT * 128], MDT,
                          kind="ExternalInput")
    if MSG_FP8:
        ipair = nc.dram_tensor("ipair", [128, 256], FP8,
                               kind="ExternalInput")
    xT = nc.dram_tensor("xT", [128, npad], BF16, kind="ExternalInput")
    sel = nc.dram_tensor("sel", [128, W * NOV], F32, kind="ExternalInput")
    wpack = nc.dram_tensor("wpack", [128, 128 * 10], BF16, kind="ExternalInput")
    rows = nc.dram_tensor("rows", [1, 256], BF16, kind="ExternalInput")
    vecs = nc.dram_tensor("vecs", [128, 7], F32, kind="ExternalInput")
    iota_in = nc.dram_tensor("iota128", [128, 128], F32, kind="ExternalInput")
    out = nc.dram_tensor("out", [npad, 128], F32, kind="ExternalOutput")

    with tile.TileContext(nc) as tc:
        with (
            tc.tile_pool(name="res", bufs=1) as res,
            tc.tile_pool(name="msgp", bufs=3) as msgp,
            tc.tile_pool(name="sp", bufs=3) as sp,
            tc.tile_pool(name="hxp", bufs=3) as hxp,
            tc.tile_pool(name="sqp", bufs=3) as sqp,
            tc.tile_pool(name="smallp", bufs=8) as smallp,
            tc.tile_pool(name="dram", bufs=1, space="DRAM") as dram,
            tc.tile_pool(name="hbnp", bufs=3) as hbnp,
            tc.tile_pool(name="ep", bufs=3) as ep,
            tc.tile_pool(name="t1p", bufs=3) as t1p,
            tc.tile_pool(name="up", bufs=3) as up,
            tc.tile_pool(name="outp", bufs=3) as outp,
        ):
            # ---------- resident loads ----------
            xT_sb = res.tile([128, npad], BF16)
            nc.sync.dma_start(xT_sb[:], xT.ap())
            sel_sb = res.tile([128, W * NOV], F32)
            nc.sync.dma_start(sel_sb[:], sel.ap())
            wp = res.tile([128, 128 * 10], BF16)
            nc.sync.dma_start(wp[:], wpack.ap())
            rows_sb = res.tile([1, 256], BF16)
            nc.sync.dma_start(rows_sb[:], rows.ap())
            vec = res.tile([128, 7], F32)
            nc.sync.dma_start(vec[:], vecs.ap())
            iota_sb = res.tile([128, 128], F32)
            nc.sync.dma_start(iota_sb[:], iota_in.ap())
            if MSG_FP8:
                ipair_sb = res.tile([128, 256], FP8)
                nc.sync.dma_start(ipair_sb[:], ipair.ap())

            h1n_sb = res.tile([128, npad], BF16)
            h1d_sb = res.tile([128, npad], BF16)
            ones_sb = res.tile([1, 128], BF16)
            nc.vector.memset(ones_sb[:], 1.0)
            stat_s = res.tile([128, 2 * WB], F32)   # sums (ACT accum)
            stat_q = res.tile([128, 2 * WB], F32)   # sumsq (DVE accum)
            # runtime BN-folded params
            cvec = res.tile([128, 2], F32)          # relu shift per branch
            w2n_s = res.tile([128, 128], BF16)
            w2df_s = res.tile([128, 128], BF16)
            m1s = res.tile([128, 128], BF16)
            m2s = res.tile([128, 128], BF16)

            def wslice(k):
                return wp[:, k * 128:(k + 1) * 128]

            def vcol(k):
                return vec[:, k:k + 1]

            # ---------- phase A: aggregate + first linear + stats ----------
            with (
                tc.tile_pool(name="agg_ps", bufs=2, space="PSUM") as agg_psp,
                tc.tile_pool(name="h1_ps", bufs=2, space="PSUM") as h1_psp,
            ):
              for wb in range(WB):
                w0 = 2 * wb
                msg = msgp.tile([128, 2 * TT * 128], MDT, tag="msg")
                nc.sync.dma_start(
                    msg[:, 0:TT * 128],
                    msgs.ap()[:, w0 * TT * 128:(w0 + 1) * TT * 128])
                nc.sync.dma_start(
                    msg[:, TT * 128:2 * TT * 128],
                    msgs.ap()[:, (w0 + 1) * TT * 128:(w0 + 2) * TT * 128])
                # one-hot S for overflow tiles of both windows
                S = sp.tile([128, 2 * NOV * 128], FP8 if MSG_FP8 else BF16,
                            tag="S")
                nc.vector.tensor_tensor(
                    out=S[:, :].rearrange("p (t j) -> p t j", j=128),
                    in0=iota_sb[:, :].rearrange("p (x j) -> p x j", x=1)
                        .to_broadcast([128, 2 * NOV, 128]),
                    in1=sel_sb[:, w0 * NOV:(w0 + 2) * NOV]
                        .to_broadcast([128, 2 * NOV, 128]),
                    op=OP.is_equal,
                )
                # agg psum layout: [w0_n | w1_n | w0_d | w1_d] (128 each).
                # One accumulation chain for the whole bank: first matmul
                # start=True, last stop=True; each byte is lazily zeroed on
                # its first write after start.
                agg = agg_psp.tile([128, 512], F32, tag="agg")
                first = True
                for i in range(2):
                    mbase = i * TT * 128
                    for ty in range(2):
                        dst_sl = slice((2 * ty + i) * 128,
                                       (2 * ty + i + 1) * 128)
                        tbase = mbase + ty * TPT * 128
                        last_grp = (i == 1 and ty == 1)
                        if MSG_FP8:
                            # DoubleRow: two 128-slot tiles per matmul
                            for t in range(0, KI - 1, 2):
                                a = tbase + t * 128
                                nc.tensor.matmul(
                                    agg[:, dst_sl],
                                    lhsT=msg[:, a:a + 256].rearrange(
                                        "p (t j) -> p t j", t=2),
                                    rhs=ipair_sb[:, :].rearrange(
                                        "p (t j) -> p t j", t=2),
                                    perf_mode=PM.DoubleRow,
                                    start=first, stop=False)
                                first = False
                            if KI % 2:
                                a = tbase + (KI - 1) * 128
                                nc.tensor.matmul(
                                    agg[:, dst_sl], lhsT=msg[:, a:a + 128],
                                    rhs=ipair_sb[:, 0:128],
                                    start=first, stop=False)
                                first = False
                            sbase = (i * 2 + ty) * OV * 128
                            for t in range(0, OV - 1, 2):
                                a = tbase + (KI + t) * 128
                                s = sbase + t * 128
                                nc.tensor.matmul(
                                    agg[:, dst_sl],
                                    lhsT=msg[:, a:a + 256].rearrange(
                                        "p (t j) -> p t j", t=2),
                                    rhs=S[:, s:s + 256].rearrange(
                                        "p (t j) -> p t j", t=2),
                                    perf_mode=PM.DoubleRow,
                                    start=False,
                                    stop=(last_grp and t == OV - 2))
                            if OV % 2:
                                a = tbase + (KI + OV - 1) * 128
                                s = sbase + (OV - 1) * 128
                                nc.tensor.matmul(
                                    agg[:, dst_sl], lhsT=msg[:, a:a + 128],
                                    rhs=S[:, s:s + 128],
                                    start=False, stop=last_grp)
                        else:
                            for t in range(KI):
                                nc.tensor.matmul(
                                    agg[:, dst_sl],
                                    lhsT=msg[:, tbase + t * 128:
                                             tbase + (t + 1) * 128],
                                    rhs=wslice(K_I),
                                    start=first, stop=False)
                                first = False
                            for t in range(OV):
                                scol = ((i * 2 + ty) * OV + t) * 128
                                nc.tensor.matmul(
                                    agg[:, dst_sl],
                                    lhsT=msg[:, tbase + (KI + t) * 128:
                                             tbase + (KI + t + 1) * 128],
                                    rhs=S[:, scol:scol + 128],
                                    start=False,
                                    stop=(last_grp and t == OV - 1))
                # hx = agg + x  (both branches in one op; reads PSUM -> DVE)
                hx = hxp.tile([128, 512], BF16, tag="hx")
                nc.vector.tensor_tensor(
                    out=hx[:, :].rearrange("p (b q) -> p b q", b=2),
                    in0=agg[:, :].rearrange("p (b q) -> p b q", b=2),
                    in1=xT_sb[:, w0 * 128:(w0 + 2) * 128]
                        .rearrange("p (b q) -> p b q", b=1)
                        .to_broadcast([128, 2, 256]),
                    op=OP.add,
                )
                h1 = h1_psp.tile([128, 512], F32, tag="h1")
                nc.tensor.matmul(h1[:, 0:256], lhsT=wslice(K_W1N),
                                 rhs=hx[:, 0:256], start=True, stop=False)
                nc.tensor.matmul(h1[:, 256:512], lhsT=wslice(K_W1D),
                                 rhs=hx[:, 256:512], start=False, stop=True)
                # copy psum -> resident bf16 (no bias!) + row-sums on ACT
                nsl = slice(w0 * 128, (w0 + 2) * 128)
                nc.scalar.activation(
                    h1n_sb[:, nsl], h1[:, 0:256], ACT.Identity,
                    accum_out=stat_s[:, 2 * wb:2 * wb + 1])
                nc.scalar.activation(
                    h1d_sb[:, nsl], h1[:, 256:512], ACT.Identity,
                    accum_out=stat_s[:, 2 * wb + 1:2 * wb + 2])
                # sum of squares from the bf16 copies on DVE (2x mode)
                sq = sqp.tile([128, 512], BF16, tag="sq")
                if USE_TTR:
                    nc.vector.tensor_tensor_reduce(
                        out=sq[:, 0:256], in0=h1n_sb[:, nsl],
                        in1=h1n_sb[:, nsl],
                        scale=1.0, scalar=0.0, op0=OP.mult, op1=OP.add,
                        accum_out=stat_q[:, 2 * wb:2 * wb + 1])
                    nc.vector.tensor_tensor_reduce(
                        out=sq[:, 256:512], in0=h1d_sb[:, nsl],
                        in1=h1d_sb[:, nsl],
                        scale=1.0, scalar=0.0, op0=OP.mult, op1=OP.add,
                        accum_out=stat_q[:, 2 * wb + 1:2 * wb + 2])
                else:
                    nc.vector.tensor_tensor(sq[:, 0:256], h1n_sb[:, nsl],
                                            h1n_sb[:, nsl], op=OP.mult)
                    nc.vector.tensor_tensor(sq[:, 256:512], h1d_sb[:, nsl],
                                            h1d_sb[:, nsl], op=OP.mult)
                    nc.vector.reduce_sum(
                        out=stat_q[:, 2 * wb:2 * wb + 1],
                        in_=sq[:, 0:256], axis=AX.X)
                    nc.vector.reduce_sum(
                        out=stat_q[:, 2 * wb + 1:2 * wb + 2],
                        in_=sq[:, 256:512], axis=AX.X)

            # ---------- stats reduce + allreduce + BN params ----------
            sums = smallp.tile([128, 4], F32, tag="sums")
            # col order: [sum_n, sumsq_n, sum_d, sumsq_d]
            for br in range(2):
                nc.vector.reduce_sum(
                    out=sums[:, 2 * br:2 * br + 1],
                    in_=stat_s[:, :].rearrange("p (w k) -> p w k", k=2)
                        [:, :, br],
                    axis=AX.X)
                nc.vector.reduce_sum(
                    out=sums[:, 2 * br + 1:2 * br + 2],
                    in_=stat_q[:, :].rearrange("p (w k) -> p w k", k=2)
                        [:, :, br],
                    axis=AX.X)
            cc_in = dram.tile([128, 4], F32)
            cc_out = dram.tile([128, 4], F32)
            nc.gpsimd.dma_start(cc_in[:], sums[:])
            nc.gpsimd.collective_compute(
                "AllReduce", OP.add,
                replica_groups=[list(range(cfg.C))],
                ins=[cc_in[:].opt()], outs=[cc_out[:].opt()],
            )
            gsums = smallp.tile([128, 4], F32, tag="gsums")
            nc.gpsimd.dma_start(gsums[:], cc_out[:])

            inv_n = 1.0 / cfg.N
            for br, (b1c, g_col, b_col) in enumerate([
                (V_B1N, V_GN, V_BN),
                (V_B1D, V_GD, V_BD),
            ]):
                mean = smallp.tile([128, 1], F32, tag="mean")
                nc.vector.tensor_scalar(
                    out=mean[:], in0=gsums[:, 2 * br:2 * br + 1],
                    scalar1=inv_n, scalar2=None, op0=OP.mult)
                ex2 = smallp.tile([128, 1], F32, tag="ex2")
                nc.vector.tensor_scalar(
                    out=ex2[:], in0=gsums[:, 2 * br + 1:2 * br + 2],
                    scalar1=inv_n, scalar2=None, op0=OP.mult)
                var = smallp.tile([128, 1], F32, tag="var")
                nc.vector.tensor_tensor(var[:], mean[:], mean[:], op=OP.mult)
                nc.vector.tensor_tensor(var[:], ex2[:], var[:],
                                        op=OP.subtract)
                # rstd = 1/sqrt(var + eps); scale = gamma * rstd  (> 0)
                nc.vector.tensor_scalar(out=var[:], in0=var[:],
                                        scalar1=BN_EPS, scalar2=None,
                                        op0=OP.add)
                std = smallp.tile([128, 1], F32, tag="std")
                nc.scalar.activation(std[:], var[:], ACT.Sqrt)
                rstd = smallp.tile([128, 1], F32, tag="rstd")
                nc.vector.reciprocal(rstd[:], std[:])
                sc = smallp.tile([128, 1], F32, tag="sc")
                nc.vector.tensor_tensor(sc[:], vcol(g_col), rstd[:],
                                        op=OP.mult)
                # c = beta / scale - mean  (the w1 bias cancels inside BN)
                rsc = smallp.tile([128, 1], F32, tag="rsc")
                nc.vector.reciprocal(rsc[:], sc[:])
                bos = smallp.tile([128, 1], F32, tag="bos")
                nc.vector.tensor_tensor(bos[:], vcol(b_col), rsc[:],
                                        op=OP.mult)
                nc.vector.tensor_tensor(cvec[:, br:br + 1], bos[:], mean[:],
                                        op=OP.subtract)
                # fold scale into second-layer weights
                wsl2 = wslice(K_W2N) if br == 0 else wslice(K_W2DF)
                wdst = w2n_s if br == 0 else w2df_s
                nc.vector.tensor_scalar(out=wdst[:], in0=wsl2,
                                        scalar1=sc[:], scalar2=None,
                                        op0=OP.mult)
                msl = wslice(K_M1T) if br == 0 else wslice(K_M2T)
                mdst = m1s if br == 0 else m2s
                nc.vector.tensor_scalar(out=mdst[:], in0=msl,
                                        scalar1=sc[:], scalar2=None,
                                        op0=OP.mult)

            # ---------- phase C: BN/relu, gate, outputs ----------
            with (
                tc.tile_pool(name="z_ps", bufs=2, space="PSUM") as z_psp,
                tc.tile_pool(name="nm_ps", bufs=2, space="PSUM") as nm_psp,
            ):
              for wb in range(WB):
                w0 = 2 * wb
                nsl = slice(w0 * 128, (w0 + 2) * 128)
                hbn = hbnp.tile([128, 512], BF16, tag="hbn")
                heng = nc.gpsimd if USE_POOL_HBN else nc.vector
                heng.tensor_scalar(
                    out=hbn[:, 0:256], in0=h1n_sb[:, nsl],
                    scalar1=cvec[:, 0:1], scalar2=0.0,
                    op0=OP.add, op1=OP.max)
                heng.tensor_scalar(
                    out=hbn[:, 256:512], in0=h1d_sb[:, nsl],
                    scalar1=cvec[:, 1:2], scalar2=0.0,
                    op0=OP.add, op1=OP.max)
                # gate logits via composed weights (bank-padded psum tile)
                z = z_psp.tile([128, 512], F32, tag="z")
                nc.tensor.matmul(z[:, 0:256], lhsT=wslice(K_M0T),
                                 rhs=xT_sb[:, nsl], start=True, stop=False)
                nc.tensor.matmul(z[:, 0:256], lhsT=m1s[:], rhs=hbn[:, 0:256],
                                 start=False, stop=False)
                nc.tensor.matmul(z[:, 0:256], lhsT=m2s[:],
                                 rhs=hbn[:, 256:512],
                                 start=False, stop=True)
                e = ep.tile([128, 256], BF16, tag="e")
                nc.scalar.activation(e[:], z[:, 0:256], ACT.Exp,
                                     bias=vcol(V_BZ))
                # nm: one psum bank per window: [ct | At | xd | pad], one
                # accumulation chain per bank.
                nm = nm_psp.tile([128, 1024], F32, tag="nm")
                for i in range(2):
                    b = i * 512
                    isl = slice((w0 + i) * 128, (w0 + i + 1) * 128)
                    nc.tensor.matmul(nm[:, b:b + 128],
                                     lhsT=e[:, i * 128:(i + 1) * 128],
                                     rhs=wslice(K_U), start=True, stop=False)
                    nc.tensor.matmul(nm[:, b + 128:b + 256],
                                     lhsT=xT_sb[:, isl], rhs=wslice(K_WSL),
                                     start=False, stop=False)
                    nc.tensor.matmul(nm[:, b + 128:b + 256],
                                     lhsT=hbn[:, i * 128:(i + 1) * 128],
                                     rhs=w2n_s[:], start=False, stop=False)
                    nc.tensor.matmul(nm[:, b + 256:b + 384],
                                     lhsT=hbn[:, 256 + i * 128:
                                              256 + (i + 1) * 128],
                                     rhs=w2df_s[:], start=False, stop=False)
                    # bias rank-1 spanning At|xd, closes the chain
                    nc.tensor.matmul(nm[:, b + 128:b + 384],
                                     lhsT=ones_sb[:], rhs=rows_sb[:, 0:256],
                                     start=False, stop=True)
                nmv = nm[:, :].rearrange("p (i q) -> p i q", q=512)
                r = smallp.tile([128, 2], F32, tag="r")
                nc.vector.reciprocal(
                    r[:, :].rearrange("p (i u) -> p i u", u=1),
                    nmv[:, :, 127:128])
                t1 = t1p.tile([128, 256], BF16, tag="t1")
                for i in range(2):
                    if USE_ACT_T1:
                        nc.scalar.activation(
                            t1[:, i * 128:(i + 1) * 128],
                            nm[:, i * 512:i * 512 + 128],
                            ACT.Identity, scale=r[:, i:i + 1])
                    else:
                        nc.vector.tensor_scalar(
                            out=t1[:, i * 128:(i + 1) * 128],
                            in0=nm[:, i * 512:i * 512 + 128],
                            scalar1=r[:, i:i + 1], scalar2=None,
                            op0=OP.mult)
                u = up.tile([128, 256], BF16, tag="u")
                nc.vector.tensor_tensor(
                    out=u[:, :].rearrange("p (i q) -> p i q", i=2),
                    in0=t1[:, :].rearrange("p (i q) -> p i q", i=2),
                    in1=nmv[:, :, 256:384], op=OP.mult)
                o = outp.tile([128, 256], F32, tag="o")
                nc.vector.tensor_tensor(
                    out=o[:, :].rearrange("p (i q) -> p i q", i=2),
                    in0=u[:, :].rearrange("p (i q) -> p i q", i=2),
                    in1=nmv[:, :, 128:256], op=OP.add)
                nc.sync.dma_start(
                    out.ap()[w0 * 128:(w0 + 2) * 128, :]
                       .rearrange("(i p) f -> p i f", i=2),
                    o[:, :].rearrange("p (i f) -> p i f", i=2))

    nc.compile()
    return nc


def pack_edges(cfg: Cfg, src, dst, et):
    """Slot assignment. Returns (off [128, W*TT] int32 per core list,
    sel [128, W*NOV] f32 per core list). Raises if OV capacity exceeded."""
    C, W, KI, OV, TPT, TT, npc = (cfg.C, cfg.W, cfg.KI, cfg.OV, cfg.TPT,
                                  cfg.TT, cfg.npc)
    NOV = 2 * OV
    E = src.shape[0]
    core = dst // npc
    ldst = dst - core * npc
    wdw = ldst >> 7
    j = ldst & 127

    # stable sort by (core, window, type, j)
    gkey = ((core.astype(np.int64) * W + wdw) * 2 + et)
    fkey = gkey * 128 + j
    order = np.argsort(fkey, kind="stable")
    fs = fkey[order]
    gs = gkey[order]
    js = j[order]
    srcs = src[order]

    # rank within (c,w,t,j)
    fcounts = np.bincount(fs, minlength=cfg.C * W * 2 * 128)
    fstarts = np.concatenate([[0], np.cumsum(fcounts)[:-1]])
    rank = np.arange(E, dtype=np.int64) - fstarts[fs]

    id_mask = rank < KI
    ov_mask = ~id_mask
    # overflow rank within (c,w,t)
    cum = np.cumsum(ov_mask)
    gcounts = np.bincount(gs, minlength=cfg.C * W * 2)
    gstarts = np.concatenate([[0], np.cumsum(gcounts)[:-1]])
    cum_at_start = np.where(gstarts > 0, cum[gstarts - 1], 0)
    ovr = cum - 1 - cum_at_start[gs]

    max_ov = int((ovr[ov_mask].max() + 1) if ov_mask.any() else 0)
    if max_ov > OV * 128:
        raise RuntimeError(f"overflow capacity exceeded: {max_ov} > {OV*128}")

    cores_s = (gs // (2 * W)).astype(np.int64)
    w_s = (gs // 2) % W
    ty_s = gs % 2

    # tile index within window and partition
    tile_idx = np.where(id_mask, rank, KI + (ovr >> 7))
    part = np.where(id_mask, js, ovr & 127)
    col = w_s * TT + ty_s * TPT + tile_idx

    ZROW = cfg.N
    off = np.full((C, 128, W * TT), ZROW, np.int32)
    off[cores_s, part, col] = srcs

    sel = np.full((C, 128, W * NOV), -1.0, np.float32)
    ov_idx = np.nonzero(ov_mask)[0]
    scol = (w_s[ov_idx] * NOV + ty_s[ov_idx] * OV
            + (ovr[ov_idx] >> 7))
    sel[cores_s[ov_idx], ovr[ov_idx] & 127, scol] = js[ov_idx].astype(
        np.float32)
    return off, sel


def prep_inputs(cfg: Cfg, x, edge_index, edge_type, w_sl, b_sl,
                w1_n, b1_n, gamma_n, beta_n, w2_n, b2_n,
                w1_d, b1_d, gamma_d, beta_d, w2_d, b2_d,
                w_gat, b_gat):
    C, npc, npad = cfg.C, cfg.npc, cfg.npad
    x = np.asarray(x, np.float32)
    src = np.asarray(edge_index[0], np.int64).astype(np.int64)
    dst = np.asarray(edge_index[1], np.int64).astype(np.int64)
    et = np.asarray(edge_type, np.int64).astype(np.int64)

    off, sel = pack_edges(cfg, src, dst, et)

    xbf = np.vstack([x, np.zeros((1, 128), np.float32)]).astype(
        F8 if MSG_FP8 else BF)
    msgs = [np.ascontiguousarray(xbf[off[c]].reshape(128, -1))
            for c in range(C)]

    xTs = []
    for c in range(C):
        xp = np.zeros((npad, 128), np.float32)
        xp[:npc] = x[c * npc:(c + 1) * npc]
        xTs.append(np.ascontiguousarray(xp.T).astype(BF))

    def bt(a):
        return np.ascontiguousarray(np.asarray(a, np.float64)).astype(BF)

    w_sl64 = np.asarray(w_sl, np.float64)
    w2n64 = np.asarray(w2_n, np.float64)
    w2d64 = np.asarray(w2_d, np.float64)
    wg = np.asarray(w_gat, np.float64)
    wg0, wg1, wg2 = wg[:, 0:128], wg[:, 128:256], wg[:, 256:384]

    wcols = [
        bt(w_sl64.T), bt(np.asarray(w1_n).T), bt(np.asarray(w1_d).T),
        bt(w2n64.T), bt(w2d64[::-1, :].T),
        bt((wg0 @ w_sl64).T), bt((wg1 @ w2n64).T), bt((wg2 @ w2d64).T),
        bt(np.triu(np.ones((128, 128), np.float32))),
        bt(np.eye(128, dtype=np.float32)),
    ]
    wpack = np.concatenate(wcols, axis=1)

    rows = np.concatenate([
        (np.asarray(b_sl, np.float64) + np.asarray(b2_n, np.float64))[None, :],
        np.asarray(b2_d, np.float64)[::-1][None, :],
    ], axis=1).astype(BF)

    bz = (np.asarray(b_gat, np.float64) + wg0 @ np.asarray(b_sl, np.float64)
          + wg1 @ np.asarray(b2_n, np.float64)
          + wg2 @ np.asarray(b2_d, np.float64))
    vecs = np.stack([
        np.asarray(b1_n, np.float64), np.asarray(b1_d, np.float64),
        np.asarray(gamma_n, np.float64), np.asarray(beta_n, np.float64),
        np.asarray(gamma_d, np.float64), np.asarray(beta_d, np.float64),
        bz,
    ], axis=1).astype(np.float32)

    in_maps = []
    for c in range(C):
        m = {
            "msgs": msgs[c],
            "xT": xTs[c],
            "sel": np.ascontiguousarray(sel[c]),
            "wpack": wpack,
            "rows": rows,
            "vecs": vecs,
            "iota128": np.broadcast_to(
                np.arange(128, dtype=np.float32)[None, :],
                (128, 128)).copy(),
        }
        if MSG_FP8:
            m["ipair"] = np.concatenate(
                [np.eye(128, dtype=np.float32)] * 2, axis=1).astype(F8)
        in_maps.append(m)
    return in_maps


_BUILD_CACHE = {}


def run(cfg: Cfg, inputs: dict, **run_kwargs):
    in_maps = None
    while True:
        try:
            in_maps = prep_inputs(cfg, **inputs)
            break
        except RuntimeError:
            cfg = Cfg(cfg.N, cfg.E, cfg.C, cfg.KI, cfg.OV + 1)
    key = (cfg.N, cfg.E, cfg.C, cfg.KI, cfg.OV,
           USE_TTR, USE_POOL_HBN, USE_ACT_T1, MSG_FP8)
    if key not in _BUILD_CACHE:
        _BUILD_CACHE[key] = build(cfg)
    nc = _BUILD_CACHE[key]
    res = run_bass_kernel_spmd(nc, in_maps, core_ids=list(range(cfg.C)),
                               **run_kwargs)
    outs = [res.results[c]["out"][:cfg.npc] for c in range(cfg.C)]
    return np.concatenate(outs, axis=0).astype(np.float32), res


def kernel(**inputs):
    out, _ = run(CFG, inputs)
    return out


# revision 33
# speedup vs baseline: 1.7446x; 1.7446x over previous
"""Trainium2 Bass kernel for a 2-relation GIN-style GNN message-passing layer.

Full (unsharded) inputs in, full output out. Internally:
  - nodes sharded across 8 NeuronCores (12500/core, padded to 12544 = 98
    windows of 128); edges partitioned by destination-node shard (CPU prep).
  - per (window, relation), edges are packed into fixed tiles of 128 slots:
      * KI "identity" tiles: the t-th edge of destination j sits in
        partition j of tile t, so segment-sum over a tile is a plain
        transpose-accumulate: matmul(lhsT=msg_tile, rhs=I128). Empty slots
        gather a zero row of x.
      * OV "overflow" one-hot tiles for edges beyond KI per destination:
        matmul with a one-hot scatter matrix S built on-device via is_equal
        (padding slots sel=-1 give zero columns).
  - per-edge source rows are pre-gathered on CPU into a bf16 stream so the
    device sees only contiguous DMA.
  - BatchNorm batch stats are computed bias-free (bias folded analytically
    into the post-BN shift), via fused copy+row-sum (scalar engine
    accumulate) and fused square+reduce (DVE tensor_tensor_reduce), and
    AllReduce'd across the 8 cores in-kernel.
  - the BN scale is folded into the second-layer weights at runtime
    (requires gamma > 0, true for this model), so BN+ReLU is a single
    add+max op.
  - gate logits are computed with CPU-composed weights
    (w_gat_chunk @ w_branch), skipping the feature-major x_new_* tensors
    entirely; cumsum = matmul with triangular ones; flip folded into
    reversed weight rows; node-major outputs via data-stationary matmuls.
"""

import numpy as np
import ml_dtypes

import concourse.bass as bass
import concourse.mybir as mybir
import concourse.tile as tile
from concourse import bacc
from concourse.bass_utils import run_bass_kernel_spmd

F32 = mybir.dt.float32
BF16 = mybir.dt.bfloat16
FP8 = mybir.dt.float8e4
AX = mybir.AxisListType
OP = mybir.AluOpType
ACT = mybir.ActivationFunctionType
PM = mybir.MatmulPerfMode

BF = ml_dtypes.bfloat16
F8 = ml_dtypes.float8_e4m3


class Cfg:
    def __init__(self, N, E, C, KI, OV):
        self.N = N            # total nodes
        self.E = E            # total edges
        self.C = C            # cores
        self.F = 128
        self.KI = KI          # identity tiles per (window, type)
        self.OV = OV          # one-hot overflow tiles per (window, type)
        self.TPT = KI + OV    # tiles per type
        self.TT = 2 * self.TPT  # tiles per window (both types)
        assert N % C == 0
        self.npc = N // C                      # real nodes per core
        self.W = (self.npc + 127) // 128       # windows per core
        assert self.W % 2 == 0
        self.WB = self.W // 2                  # 2-window iterations
        self.npad = self.W * 128               # padded nodes per core


CFG = Cfg(N=100000, E=1600000, C=8, KI=8, OV=2)

# column layout of the "vecs" [128, 7] f32 input
(V_B1N, V_B1D, V_GN, V_BN, V_GD, V_BD, V_BZ) = range(7)

# column layout of wpack [128, 128*10] bf16
(K_WSL, K_W1N, K_W1D, K_W2N, K_W2DF, K_M0T, K_M1T, K_M2T, K_U, K_I) = range(10)

BN_EPS = 1e-5


USE_TTR = False       # tensor_tensor_reduce hangs TRN2 HW via this path
USE_POOL_HBN = False  # gpsimd tensor ops are ~8x slower than modeled
USE_ACT_T1 = True     # t1 via ACT Identity+scale (else DVE tensor_scalar)
MSG_FP8 = False       # fp8e4m3 message stream + DoubleRow paired matmuls


def build(cfg: Cfg):
    nc = bacc.Bacc("TRN2", target_bir_lowering=False, debug=False,
                   num_devices=cfg.C)
    W, WB, KI, OV, TPT, TT, npad = (cfg.W, cfg.WB, cfg.KI, cfg.OV,
                                    cfg.TPT, cfg.TT, cfg.npad)
    NOV = 2 * OV   # overflow tiles per window (both types)

    MDT = FP8 if MSG_FP8 else BF16
    msgs = nc.dram_tensor("msgs", [128, W * TT * 128], MDT,
                          kind="ExternalInput")
    if MSG_FP8:
        ipair = nc.dram_tensor("ipair", [128, 256], FP8,
                               kind="ExternalInput")
    xT = nc.dram_tensor("xT", [128, npad], BF16, kind="ExternalInput")
    sel = nc.dram_tensor("sel", [128, W * NOV], F32, kind="ExternalInput")
    wpack = nc.dram_tensor("wpack", [128, 128 * 10], BF16, kind="ExternalInput")
    rows = nc.dram_tensor("rows", [1, 256], BF16, kind="ExternalInput")
    vecs = nc.dram_tensor("vecs", [128, 7], F32, kind="ExternalInput")
    iota_in = nc.dram_tensor("iota128", [128, 128], F32, kind="ExternalInput")
    out = nc.dram_tensor("out", [npad, 128], F32, kind="ExternalOutput")

    with tile.TileContext(nc) as tc:
        with (
            tc.tile_pool(name="res", bufs=1) as res,
            tc.tile_pool(name="msgp", bufs=3) as msgp,
            tc.tile_pool(name="sp", bufs=3) as sp,
            tc.tile_pool(name="hxp", bufs=3) as hxp,
            tc.tile_pool(name="sqp", bufs=3) as sqp,
            tc.tile_pool(name="smallp", bufs=8) as smallp,
            tc.tile_pool(name="dram", bufs=1, space="DRAM") as dram,
            tc.tile_pool(name="hbnp", bufs=3) as hbnp,
            tc.tile_pool(name="ep", bufs=3) as ep,
            tc.tile_pool(name="t1p", bufs=3) as t1p,
            tc.tile_pool(name="up", bufs=3) as up,
            tc.tile_pool(name="outp", bufs=3) as outp,
        ):
            # ---------- resident loads ----------
            xT_sb = res.tile([128, npad], BF16)
            nc.sync.dma_start(xT_sb[:], xT.ap())
            sel_sb = res.tile([128, W * NOV], F32)
            nc.sync.dma_start(sel_sb[:], sel.ap())
            wp = res.tile([128, 128 * 10], BF16)
            nc.sync.dma_start(wp[:], wpack.ap())
            rows_sb = res.tile([1, 256], BF16)
            nc.sync.dma_start(rows_sb[:], rows.ap())
            vec = res.tile([128, 7], F32)
            nc.sync.dma_start(vec[:], vecs.ap())
            iota_sb = res.tile([128, 128], F32)
            nc.sync.dma_start(iota_sb[:], iota_in.ap())
            if MSG_FP8:
                ipair_sb = res.tile([128, 256], FP8)
                nc.sync.dma_start(ipair_sb[:], ipair.ap())

            h1n_sb = res.tile([128, npad], BF16)
            h1d_sb = res.tile([128, npad], BF16)
            ones_sb = res.tile([1, 128], BF16)
            nc.vector.memset(ones_sb[:], 1.0)
            stat_s = res.tile([128, 2 * WB], F32)   # sums (ACT accum)
            stat_q = res.tile([128, 2 * WB], F32)   # sumsq (DVE accum)
            # runtime BN-folded params
            cvec = res.tile([128, 2], F32)          # relu shift per branch
            w2n_s = res.tile([128, 128], BF16)
            w2df_s = res.tile([128, 128], BF16)
            m1s = res.tile([128, 128], BF16)
            m2s = res.tile([128, 128], BF16)

            def wslice(k):
                return wp[:, k * 128:(k + 1) * 128]

            def vcol(k):
                return vec[:, k:k + 1]

            # ---------- phase A: aggregate + first linear + stats ----------
            with (
                tc.tile_pool(name="agg_ps", bufs=2, space="PSUM") as agg_psp,
                tc.tile_pool(name="h1_ps", bufs=2, space="PSUM") as h1_psp,
            ):
              for wb in range(WB):
                w0 = 2 * wb
                msg = msgp.tile([128, 2 * TT * 128], MDT, tag="msg")
                nc.sync.dma_start(
                    msg[:, :],
                    msgs.ap()[:, w0 * TT * 128:(w0 + 2) * TT * 128])
                # one-hot S for overflow tiles of both windows
                S = sp.tile([128, 2 * NOV * 128], FP8 if MSG_FP8 else BF16,
                            tag="S")
                nc.vector.tensor_tensor(
                    out=S[:, :].rearrange("p (t j) -> p t j", j=128),
                    in0=iota_sb[:, :].rearrange("p (x j) -> p x j", x=1)
                        .to_broadcast([128, 2 * NOV, 128]),
                    in1=sel_sb[:, w0 * NOV:(w0 + 2) * NOV]
                        .to_broadcast([128, 2 * NOV, 128]),
                    op=OP.is_equal,
                )
                # agg psum layout: [w0_n | w1_n | w0_d | w1_d] (128 each).
                # One accumulation chain for the whole bank: first matmul
                # start=True, last stop=True; each byte is lazily zeroed on
                # its first write after start.
                agg = agg_psp.tile([128, 512], F32, tag="agg")
                first = True
                for i in range(2):
                    mbase = i * TT * 128
                    for ty in range(2):
                        dst_sl = slice((2 * ty + i) * 128,
                                       (2 * ty + i + 1) * 128)
                        tbase = mbase + ty * TPT * 128
                        last_grp = (i == 1 and ty == 1)
                        if MSG_FP8:
                            # DoubleRow: two 128-slot tiles per matmul
                            for t in range(0, KI - 1, 2):
                                a = tbase + t * 128
                                nc.tensor.matmul(
                                    agg[:, dst_sl],
                                    lhsT=msg[:, a:a + 256].rearrange(
                                        "p (t j) -> p t j", t=2),
                                    rhs=ipair_sb[:, :].rearrange(
                                        "p (t j) -> p t j", t=2),
                                    perf_mode=PM.DoubleRow,
                                    start=first, stop=False)
                                first = False
                            if KI % 2:
                                a = tbase + (KI - 1) * 128
                                nc.tensor.matmul(
                                    agg[:, dst_sl], lhsT=msg[:, a:a + 128],
                                    rhs=ipair_sb[:, 0:128],
                                    start=first, stop=False)
                                first = False
                            sbase = (i * 2 + ty) * OV * 128
                            for t in range(0, OV - 1, 2):
                                a = tbase + (KI + t) * 128
                                s = sbase + t * 128
                                nc.tensor.matmul(
                                    agg[:, dst_sl],
                                    lhsT=msg[:, a:a + 256].rearrange(
                                        "p (t j) -> p t j", t=2),
                                    rhs=S[:, s:s + 256].rearrange(
                                        "p (t j) -> p t j", t=2),
                                    perf_mode=PM.DoubleRow,
                                    start=False,
                                    stop=(last_grp and t == OV - 2))
                            if OV % 2:
                                a = tbase + (KI + OV - 1) * 128
                                s = sbase + (OV - 1) * 128
                                nc.tensor.matmul(
                                    agg[:, dst_sl], lhsT=msg[:, a:a + 128],
                                    rhs=S[:, s:s + 128],
                                    start=False, stop=last_grp)
                        else:
                            for t in range(KI):
                                nc.tensor.matmul(
                                    agg[:, dst_sl],
                                    lhsT=msg[:, tbase + t * 128:
                                             tbase + (t + 1) * 128],
                                    rhs=wslice(K_I),
                                    start=first, stop=False)
                                first = False
                            for t in range(OV):
                                scol = ((i * 2 + ty) * OV + t) * 128
                                nc.tensor.matmul(
                                    agg[:, dst_sl],
                                    lhsT=msg[:, tbase + (KI + t) * 128:
                                             tbase + (KI + t + 1) * 128],
                                    rhs=S[:, scol:scol + 128],
                                    start=False,
                                    stop=(last_grp and t == OV - 1))
                # hx = agg + x  (both branches in one op; reads PSUM -> DVE)
                hx = hxp.tile([128, 512], BF16, tag="hx")
                nc.vector.tensor_tensor(
                    out=hx[:, :].rearrange("p (b q) -> p b q", b=2),
                    in0=agg[:, :].rearrange("p (b q) -> p b q", b=2),
                    in1=xT_sb[:, w0 * 128:(w0 + 2) * 128]
                        .rearrange("p (b q) -> p b q", b=1)
                        .to_broadcast([128, 2, 256]),
                    op=OP.add,
                )
                h1 = h1_psp.tile([128, 512], F32, tag="h1")
                nc.tensor.matmul(h1[:, 0:256], lhsT=wslice(K_W1N),
                                 rhs=hx[:, 0:256], start=True, stop=False)
                nc.tensor.matmul(h1[:, 256:512], lhsT=wslice(K_W1D),
                                 rhs=hx[:, 256:512], start=False, stop=True)
                # copy psum -> resident bf16 (no bias!) + row-sums on ACT
                nsl = slice(w0 * 128, (w0 + 2) * 128)
                nc.scalar.activation(
                    h1n_sb[:, nsl], h1[:, 0:256], ACT.Identity,
                    accum_out=stat_s[:, 2 * wb:2 * wb + 1])
                nc.scalar.activation(
                    h1d_sb[:, nsl], h1[:, 256:512], ACT.Identity,
                    accum_out=stat_s[:, 2 * wb + 1:2 * wb + 2])
                # sum of squares from the bf16 copies on DVE (2x mode)
                sq = sqp.tile([128, 512], BF16, tag="sq")
                if USE_TTR:
                    nc.vector.tensor_tensor_reduce(
                        out=sq[:, 0:256], in0=h1n_sb[:, nsl],
                        in1=h1n_sb[:, nsl],
                        scale=1.0, scalar=0.0, op0=OP.mult, op1=OP.add,
                        accum_out=stat_q[:, 2 * wb:2 * wb + 1])
                    nc.vector.tensor_tensor_reduce(
                        out=sq[:, 256:512], in0=h1d_sb[:, nsl],
                        in1=h1d_sb[:, nsl],
                        scale=1.0, scalar=0.0, op0=OP.mult, op1=OP.add,
                        accum_out=stat_q[:, 2 * wb + 1:2 * wb + 2])
                else:
                    nc.vector.tensor_tensor(sq[:, 0:256], h1n_sb[:, nsl],
                                            h1n_sb[:, nsl], op=OP.mult)
                    nc.vector.tensor_tensor(sq[:, 256:512], h1d_sb[:, nsl],
                                            h1d_sb[:, nsl], op=OP.mult)
                    nc.vector.reduce_sum(
                        out=stat_q[:, 2 * wb:2 * wb + 1],
                        in_=sq[:, 0:256], axis=AX.X)
                    nc.vector.reduce_sum(
                        out=stat_q[:, 2 * wb + 1:2 * wb + 2],
                        in_=sq[:, 256:512], axis=AX.X)

            # ---------- stats reduce + allreduce + BN params ----------
            sums = smallp.tile([128, 4], F32, tag="sums")
            # col order: [sum_n, sumsq_n, sum_d, sumsq_d]
            for br in range(2):
                nc.vector.reduce_sum(
                    out=sums[:, 2 * br:2 * br + 1],
                    in_=stat_s[:, :].rearrange("p (w k) -> p w k", k=2)
                        [:, :, br],
                    axis=AX.X)
                nc.vector.reduce_sum(
                    out=sums[:, 2 * br + 1:2 * br + 2],
                    in_=stat_q[:, :].rearrange("p (w k) -> p w k", k=2)
                        [:, :, br],
                    axis=AX.X)
            cc_in = dram.tile([128, 4], F32)
            cc_out = dram.tile([128, 4], F32)
            nc.gpsimd.dma_start(cc_in[:], sums[:])
            nc.gpsimd.collective_compute(
                "AllReduce", OP.add,
                replica_groups=[list(range(cfg.C))],
                ins=[cc_in[:].opt()], outs=[cc_out[:].opt()],
            )
            gsums = smallp.tile([128, 4], F32, tag="gsums")
            nc.gpsimd.dma_start(gsums[:], cc_out[:])

            inv_n = 1.0 / cfg.N
            for br, (b1c, g_col, b_col) in enumerate([
                (V_B1N, V_GN, V_BN),
                (V_B1D, V_GD, V_BD),
            ]):
                mean = smallp.tile([128, 1], F32, tag="mean")
                nc.vector.tensor_scalar(
                    out=mean[:], in0=gsums[:, 2 * br:2 * br + 1],
                    scalar1=inv_n, scalar2=None, op0=OP.mult)
                ex2 = smallp.tile([128, 1], F32, tag="ex2")
                nc.vector.tensor_scalar(
                    out=ex2[:], in0=gsums[:, 2 * br + 1:2 * br + 2],
                    scalar1=inv_n, scalar2=None, op0=OP.mult)
                var = smallp.tile([128, 1], F32, tag="var")
                nc.vector.tensor_tensor(var[:], mean[:], mean[:], op=OP.mult)
                nc.vector.tensor_tensor(var[:], ex2[:], var[:],
                                        op=OP.subtract)
                # rstd = 1/sqrt(var + eps); scale = gamma * rstd  (> 0)
                nc.vector.tensor_scalar(out=var[:], in0=var[:],
                                        scalar1=BN_EPS, scalar2=None,
                                        op0=OP.add)
                std = smallp.tile([128, 1], F32, tag="std")
                nc.scalar.activation(std[:], var[:], ACT.Sqrt)
                rstd = smallp.tile([128, 1], F32, tag="rstd")
                nc.vector.reciprocal(rstd[:], std[:])
                sc = smallp.tile([128, 1], F32, tag="sc")
                nc.vector.tensor_tensor(sc[:], vcol(g_col), rstd[:],
                                        op=OP.mult)
                # c = beta / scale - mean  (the w1 bias cancels inside BN)
                rsc = smallp.tile([128, 1], F32, tag="rsc")
                nc.vector.reciprocal(rsc[:], sc[:])
                bos = smallp.tile([128, 1], F32, tag="bos")
                nc.vector.tensor_tensor(bos[:], vcol(b_col), rsc[:],
                                        op=OP.mult)
                nc.vector.tensor_tensor(cvec[:, br:br + 1], bos[:], mean[:],
                                        op=OP.subtract)
                # fold scale into second-layer weights
                wsl2 = wslice(K_W2N) if br == 0 else wslice(K_W2DF)
                wdst = w2n_s if br == 0 else w2df_s
                nc.vector.tensor_scalar(out=wdst[:], in0=wsl2,
                                        scalar1=sc[:], scalar2=None,
                                        op0=OP.mult)
                msl = wslice(K_M1T) if br == 0 else wslice(K_M2T)
                mdst = m1s if br == 0 else m2s
                nc.vector.tensor_scalar(out=mdst[:], in0=msl,
                                        scalar1=sc[:], scalar2=None,
                                        op0=OP.mult)

            # ---------- phase C: BN/relu, gate, outputs ----------
            with (
                tc.tile_pool(name="z_ps", bufs=2, space="PSUM") as z_psp,
                tc.tile_pool(name="nm_ps", bufs=2, space="PSUM") as nm_psp,
            ):
              for wb in range(WB):
                w0 = 2 * wb
                nsl = slice(w0 * 128, (w0 + 2) * 128)
                hbn = hbnp.tile([128, 512], BF16, tag="hbn")
                heng = nc.gpsimd if USE_POOL_HBN else nc.vector
                heng.tensor_scalar(
                    out=hbn[:, 0:256], in0=h1n_sb[:, nsl],
                    scalar1=cvec[:, 0:1], scalar2=0.0,
                    op0=OP.add, op1=OP.max)
                heng.tensor_scalar(
                    out=hbn[:, 256:512], in0=h1d_sb[:, nsl],
                    scalar1=cvec[:, 1:2], scalar2=0.0,
                    op0=OP.add, op1=OP.max)
                # gate logits via composed weights (bank-padded psum tile)
                z = z_psp.tile([128, 512], F32, tag="z")
                nc.tensor.matmul(z[:, 0:256], lhsT=wslice(K_M0T),
                                 rhs=xT_sb[:, nsl], start=True, stop=False)
                nc.tensor.matmul(z[:, 0:256], lhsT=m1s[:], rhs=hbn[:, 0:256],
                                 start=False, stop=False)
                nc.tensor.matmul(z[:, 0:256], lhsT=m2s[:],
                                 rhs=hbn[:, 256:512],
                                 start=False, stop=True)
                e = ep.tile([128, 256], BF16, tag="e")
                nc.scalar.activation(e[:], z[:, 0:256], ACT.Exp,
                                     bias=vcol(V_BZ))
                # nm: one psum bank per window: [ct | At | xd | pad], one
                # accumulation chain per bank.
                nm = nm_psp.tile([128, 1024], F32, tag="nm")
                for i in range(2):
                    b = i * 512
                    isl = slice((w0 + i) * 128, (w0 + i + 1) * 128)
                    nc.tensor.matmul(nm[:, b:b + 128],
                                     lhsT=e[:, i * 128:(i + 1) * 128],
                                     rhs=wslice(K_U), start=True, stop=False)
                    nc.tensor.matmul(nm[:, b + 128:b + 256],
                                     lhsT=xT_sb[:, isl], rhs=wslice(K_WSL),
                                     start=False, stop=False)
                    nc.tensor.matmul(nm[:, b + 128:b + 256],
                                     lhsT=hbn[:, i * 128:(i + 1) * 128],
                                     rhs=w2n_s[:], start=False, stop=False)
                    nc.tensor.matmul(nm[:, b + 256:b + 384],
                                     lhsT=hbn[:, 256 + i * 128:
                                              256 + (i + 1) * 128],
                                     rhs=w2df_s[:], start=False, stop=False)
                    # bias rank-1 spanning At|xd, closes the chain
                    nc.tensor.matmul(nm[:, b + 128:b + 384],
                                     lhsT=ones_sb[:], rhs=rows_sb[:, 0:256],
                                     start=False, stop=True)
                nmv = nm[:, :].rearrange("p (i q) -> p i q", q=512)
                r = smallp.tile([128, 2], F32, tag="r")
                nc.vector.reciprocal(
                    r[:, :].rearrange("p (i u) -> p i u", u=1),
                    nmv[:, :, 127:128])
                t1 = t1p.tile([128, 256], BF16, tag="t1")
                for i in range(2):
                    if USE_ACT_T1:
                        nc.scalar.activation(
                            t1[:, i * 128:(i + 1) * 128],
                            nm[:, i * 512:i * 512 + 128],
                            ACT.Identity, scale=r[:, i:i + 1])
                    else:
                        nc.vector.tensor_scalar(
                            out=t1[:, i * 128:(i + 1) * 128],
                            in0=nm[:, i * 512:i * 512 + 128],
                            scalar1=r[:, i:i + 1], scalar2=None,
                            op0=OP.mult)
                u = up.tile([128, 256], BF16, tag="u")
                nc.vector.tensor_tensor(
                    out=u[:, :].rearrange("p (i q) -> p i q", i=2),
                    in0=t1[:, :].rearrange("p (i q) -> p i q", i=2),
                    in1=nmv[:, :, 256:384], op=OP.mult)
                o = outp.tile([128, 256], F32, tag="o")
                nc.vector.tensor_tensor(
                    out=o[:, :].rearrange("p (i q) -> p i q", i=2),
                    in0=u[:, :].rearrange("p (i q) -> p i q", i=2),
                    in1=nmv[:, :, 128:256], op=OP.add)
                nc.sync.dma_start(
                    out.ap()[w0 * 128:(w0 + 2) * 128, :]
                       .rearrange("(i p) f -> p i f", i=2),
                    o[:, :].rearrange("p (i f) -> p i f", i=2))

    nc.compile()
    return nc


def pack_edges(cfg: Cfg, src, dst, et):
    """Slot assignment. Returns (off [128, W*TT] int32 per core list,
    sel [128, W*NOV] f32 per core list). Raises if OV capacity exceeded."""
    C, W, KI, OV, TPT, TT, npc = (cfg.C, cfg.W, cfg.KI, cfg.OV, cfg.TPT,
                                  cfg.TT, cfg.npc)
    NOV = 2 * OV
    E = src.shape[0]
    core = dst // npc
    ldst = dst - core * npc
    wdw = ldst >> 7
    j = ldst & 127

    # stable sort by (core, window, type, j)
    gkey = ((core.astype(np.int64) * W + wdw) * 2 + et)
    fkey = gkey * 128 + j
    order = np.argsort(fkey, kind="stable")
    fs = fkey[order]
    gs = gkey[order]
    js = j[order]
    srcs = src[order]

    # rank within (c,w,t,j)
    fcounts = np.bincount(fs, minlength=cfg.C * W * 2 * 128)
    fstarts = np.concatenate([[0], np.cumsum(fcounts)[:-1]])
    rank = np.arange(E, dtype=np.int64) - fstarts[fs]

    id_mask = rank < KI
    ov_mask = ~id_mask
    # overflow rank within (c,w,t)
    cum = np.cumsum(ov_mask)
    gcounts = np.bincount(gs, minlength=cfg.C * W * 2)
    gstarts = np.concatenate([[0], np.cumsum(gcounts)[:-1]])
    cum_at_start = np.where(gstarts > 0, cum[gstarts - 1], 0)
    ovr = cum - 1 - cum_at_start[gs]

    max_ov = int((ovr[ov_mask].max() + 1) if ov_mask.any() else 0)
    if max_ov > OV * 128:
        raise RuntimeError(f"overflow capacity exceeded: {max_ov} > {OV*128}")

    cores_s = (gs // (2 * W)).astype(np.int64)
    w_s = (gs // 2) % W
    ty_s = gs % 2

    # tile index within window and partition
    tile_idx = np.where(id_mask, rank, KI + (ovr >> 7))
    part = np.where(id_mask, js, ovr & 127)
    col = w_s * TT + ty_s * TPT + tile_idx

    ZROW = cfg.N
    off = np.full((C, 128, W * TT), ZROW, np.int32)
    off[cores_s, part, col] = srcs

    sel = np.full((C, 128, W * NOV), -1.0, np.float32)
    ov_idx = np.nonzero(ov_mask)[0]
    scol = (w_s[ov_idx] * NOV + ty_s[ov_idx] * OV
            + (ovr[ov_idx] >> 7))
    sel[cores_s[ov_idx], ovr[ov_idx] & 127, scol] = js[ov_idx].astype(
        np.float32)
    return off, sel


def prep_inputs(cfg: Cfg, x, edge_index, edge_type, w_sl, b_sl,
                w1_n, b1_n, gamma_n, beta_n, w2_n, b2_n,
                w1_d, b1_d, gamma_d, beta_d, w2_d, b2_d,
                w_gat, b_gat):
    C, npc, npad = cfg.C, cfg.npc, cfg.npad
    x = np.asarray(x, np.float32)
    src = np.asarray(edge_index[0], np.int64).astype(np.int64)
    dst = np.asarray(edge_index[1], np.int64).astype(np.int64)
    et = np.asarray(edge_type, np.int64).astype(np.int64)

    off, sel = pack_edges(cfg, src, dst, et)

    xbf = np.vstack([x, np.zeros((1, 128), np.float32)]).astype(
        F8 if MSG_FP8 else BF)
    msgs = [np.ascontiguousarray(xbf[off[c]].reshape(128, -1))
            for c in range(C)]

    xTs = []
    for c in range(C):
        xp = np.zeros((npad, 128), np.float32)
        xp[:npc] = x[c * npc:(c + 1) * npc]
        xTs.append(np.ascontiguousarray(xp.T).astype(BF))

    def bt(a):
        return np.ascontiguousarray(np.asarray(a, np.float64)).astype(BF)

    w_sl64 = np.asarray(w_sl, np.float64)
    w2n64 = np.asarray(w2_n, np.float64)
    w2d64 = np.asarray(w2_d, np.float64)
    wg = np.asarray(w_gat, np.float64)
    wg0, wg1, wg2 = wg[:, 0:128], wg[:, 128:256], wg[:, 256:384]

    wcols = [
        bt(w_sl64.T), bt(np.asarray(w1_n).T), bt(np.asarray(w1_d).T),
        bt(w2n64.T), bt(w2d64[::-1, :].T),
        bt((wg0 @ w_sl64).T), bt((wg1 @ w2n64).T), bt((wg2 @ w2d64).T),
        bt(np.triu(np.ones((128, 128), np.float32))),
        bt(np.eye(128, dtype=np.float32)),
    ]
    wpack = np.concatenate(wcols, axis=1)

    rows = np.concatenate([
        (np.asarray(b_sl, np.float64) + np.asarray(b2_n, np.float64))[None, :],
        np.asarray(b2_d, np.float64)[::-1][None, :],
    ], axis=1).astype(BF)

    bz = (np.asarray(b_gat, np.float64) + wg0 @ np.asarray(b_sl, np.float64)
          + wg1 @ np.asarray(b2_n, np.float64)
          + wg2 @ np.asarray(b2_d, np.float64))
    vecs = np.stack([
        np.asarray(b1_n, np.float64), np.asarray(b1_d, np.float64),
        np.asarray(gamma_n, np.float64), np.asarray(beta_n, np.float64),
        np.asarray(gamma_d, np.float64), np.asarray(beta_d, np.float64),
        bz,
    ], axis=1).astype(np.float32)

    in_maps = []
    for c in range(C):
        m = {
            "msgs": msgs[c],
            "xT": xTs[c],
            "sel": np.ascontiguousarray(sel[c]),
            "wpack": wpack,
            "rows": rows,
            "vecs": vecs,
            "iota128": np.broadcast_to(
                np.arange(128, dtype=np.float32)[None, :],
                (128, 128)).copy(),
        }
        if MSG_FP8:
            m["ipair"] = np.concatenate(
                [np.eye(128, dtype=np.float32)] * 2, axis=1).astype(F8)
        in_maps.append(m)
    return in_maps


_BUILD_CACHE = {}


def run(cfg: Cfg, inputs: dict, **run_kwargs):
    in_maps = None
    while True:
        try:
            in_maps = prep_inputs(cfg, **inputs)
            break
        except RuntimeError:
            cfg = Cfg(cfg.N, cfg.E, cfg.C, cfg.KI, cfg.OV + 1)
    key = (cfg.N, cfg.E, cfg.C, cfg.KI, cfg.OV,
           USE_TTR, USE_POOL_HBN, USE_ACT_T1, MSG_FP8)
    if key not in _BUILD_CACHE:
        _BUILD_CACHE[key] = build(cfg)
    nc = _BUILD_CACHE[key]
    res = run_bass_kernel_spmd(nc, in_maps, core_ids=list(range(cfg.C)),
                               **run_kwargs)
    outs = [res.results[c]["out"][:cfg.npc] for c in range(cfg.C)]
    return np.concatenate(outs, axis=0).astype(np.float32), res


def kernel(**inputs):
    out, _ = run(CFG, inputs)
    return out


# revision 46
# speedup vs baseline: 1.8379x; 1.0534x over previous
"""Trainium2 Bass kernel for a 2-relation GIN-style GNN message-passing layer.

Full (unsharded) inputs in, full output out. Internally:
  - nodes sharded across 8 NeuronCores (12500/core, padded to 12544 = 98
    windows of 128); edges partitioned by destination-node shard (CPU prep).
  - per (window, relation), edges are packed into fixed tiles of 128 slots:
      * KI "identity" tiles: the t-th edge of destination j sits in
        partition j of tile t, so segment-sum over a tile is a plain
        transpose-accumulate: matmul(lhsT=msg_tile, rhs=I128). Empty slots
        gather a zero row of x.
      * OV "overflow" one-hot tiles for edges beyond KI per destination:
        matmul with a one-hot scatter matrix S built on-device via is_equal
        (padding slots sel=-1 give zero columns).
  - per-edge source rows are pre-gathered on CPU into a bf16 stream so the
    device sees only contiguous DMA.
  - BatchNorm batch stats are computed bias-free (bias folded analytically
    into the post-BN shift), via fused copy+row-sum (scalar engine
    accumulate) and fused square+reduce (DVE tensor_tensor_reduce), and
    AllReduce'd across the 8 cores in-kernel.
  - the BN scale is folded into the second-layer weights at runtime
    (requires gamma > 0, true for this model), so BN+ReLU is a single
    add+max op.
  - gate logits are computed with CPU-composed weights
    (w_gat_chunk @ w_branch), skipping the feature-major x_new_* tensors
    entirely; cumsum = matmul with triangular ones; flip folded into
    reversed weight rows; node-major outputs via data-stationary matmuls.
"""

import numpy as np
import ml_dtypes

import concourse.bass as bass
import concourse.mybir as mybir
import concourse.tile as tile
from concourse import bacc
from concourse.bass_utils import run_bass_kernel_spmd

F32 = mybir.dt.float32
BF16 = mybir.dt.bfloat16
FP8 = mybir.dt.float8e4
AX = mybir.AxisListType
OP = mybir.AluOpType
ACT = mybir.ActivationFunctionType
PM = mybir.MatmulPerfMode

BF = ml_dtypes.bfloat16
F8 = ml_dtypes.float8_e4m3


class Cfg:
    def __init__(self, N, E, C, KI, OV):
        self.N = N            # total nodes
        self.E = E            # total edges
        self.C = C            # cores
        self.F = 128
        self.KI = KI          # identity tiles per (window, type)
        self.OV = OV          # one-hot overflow tiles per (window, type)
        self.TPT = KI + OV    # tiles per type
        self.TT = 2 * self.TPT  # tiles per window (both types)
        assert N % C == 0
        self.npc = N // C                      # real nodes per core
        self.W = (self.npc + 127) // 128       # windows per core
        assert self.W % 2 == 0
        self.WB = self.W // 2                  # 2-window iterations
        self.npad = self.W * 128               # padded nodes per core


CFG = Cfg(N=100000, E=1600000, C=8, KI=8, OV=2)

# column layout of the "vecs" [128, 7] f32 input
(V_B1N, V_B1D, V_GN, V_BN, V_GD, V_BD, V_BZ) = range(7)

# column layout of wpack [128, 128*10] bf16
(K_WSL, K_W1N, K_W1D, K_W2N, K_W2DF, K_M0T, K_M1T, K_M2T, K_U, K_I) = range(10)

BN_EPS = 1e-5


USE_TTR = False       # tensor_tensor_reduce hangs TRN2 HW via this path
USE_STT = True        # fused square+accumulate via scalar_tensor_tensor
USE_POOL_HBN = False  # gpsimd tensor ops are ~8x slower than modeled
USE_ACT_T1 = True     # t1 via ACT Identity+scale (else DVE tensor_scalar)
MSG_FP8 = False       # fp8e4m3 message stream + DoubleRow paired matmuls


def build(cfg: Cfg):
    nc = bacc.Bacc("TRN2", target_bir_lowering=False, debug=False,
                   num_devices=cfg.C)
    W, WB, KI, OV, TPT, TT, npad = (cfg.W, cfg.WB, cfg.KI, cfg.OV,
                                    cfg.TPT, cfg.TT, cfg.npad)
    NOV = 2 * OV   # overflow tiles per window (both types)

    MDT = FP8 if MSG_FP8 else BF16
    msgs = nc.dram_tensor("msgs", [128, W * TT * 128], MDT,
                          kind="ExternalInput")
    xself = nc.dram_tensor("xself", [128, npad], BF16, kind="ExternalInput")
    if MSG_FP8:
        ipair = nc.dram_tensor("ipair", [128, 256], FP8,
                               kind="ExternalInput")
    xT = nc.dram_tensor("xT", [128, npad], BF16, kind="ExternalInput")
    sel = nc.dram_tensor("sel", [128, W * NOV], F32, kind="ExternalInput")
    wpack = nc.dram_tensor("wpack", [128, 128 * 10], BF16, kind="ExternalInput")
    rows = nc.dram_tensor("rows", [1, 256], BF16, kind="ExternalInput")
    vecs = nc.dram_tensor("vecs", [128, 7], F32, kind="ExternalInput")
    iota_in = nc.dram_tensor("iota128", [128, 128], F32, kind="ExternalInput")
    out = nc.dram_tensor("out", [npad, 128], F32, kind="ExternalOutput")

    with tile.TileContext(nc) as tc:
        with (
            tc.tile_pool(name="res", bufs=1) as res,
            tc.tile_pool(name="msgp", bufs=3) as msgp,
            tc.tile_pool(name="sp", bufs=3) as sp,
            tc.tile_pool(name="hxp", bufs=3) as hxp,
            tc.tile_pool(name="sqp", bufs=3) as sqp,
            tc.tile_pool(name="smallp", bufs=8) as smallp,
            tc.tile_pool(name="dram", bufs=1, space="DRAM") as dram,
            tc.tile_pool(name="hbnp", bufs=3) as hbnp,
            tc.tile_pool(name="ep", bufs=3) as ep,
            tc.tile_pool(name="t1p", bufs=3) as t1p,
            tc.tile_pool(name="up", bufs=3) as up,
            tc.tile_pool(name="outp", bufs=3) as outp,
        ):
            # ---------- resident loads ----------
            xT_sb = res.tile([128, npad], BF16)
            nc.sync.dma_start(xT_sb[:], xT.ap())
            xself_sb = res.tile([128, npad], BF16)
            nc.sync.dma_start(xself_sb[:], xself.ap())
            sel_sb = res.tile([128, W * NOV], F32)
            nc.sync.dma_start(sel_sb[:], sel.ap())
            wp = res.tile([128, 128 * 10], BF16)
            nc.sync.dma_start(wp[:], wpack.ap())
            rows_sb = res.tile([1, 256], BF16)
            nc.sync.dma_start(rows_sb[:], rows.ap())
            vec = res.tile([128, 7], F32)
            nc.sync.dma_start(vec[:], vecs.ap())
            iota_sb = res.tile([128, 128], F32)
            nc.sync.dma_start(iota_sb[:], iota_in.ap())
            if MSG_FP8:
                ipair_sb = res.tile([128, 256], FP8)
                nc.sync.dma_start(ipair_sb[:], ipair.ap())

            h1n_sb = res.tile([128, npad], BF16)
            h1d_sb = res.tile([128, npad], BF16)
            ones_sb = res.tile([1, 128], BF16)
            nc.vector.memset(ones_sb[:], 1.0)
            stat_s = res.tile([128, 2 * WB], F32)   # sums (ACT accum)
            stat_q = res.tile([128, 2 * WB], F32)   # sumsq (DVE accum)
            # runtime BN-folded params
            cvec = res.tile([128, 2], F32)          # relu shift per branch
            w2n_s = res.tile([128, 128], BF16)
            w2df_s = res.tile([128, 128], BF16)
            m1s = res.tile([128, 128], BF16)
            m2s = res.tile([128, 128], BF16)

            def wslice(k):
                return wp[:, k * 128:(k + 1) * 128]

            def vcol(k):
                return vec[:, k:k + 1]

            # ---------- phase A: aggregate + first linear + stats ----------
            with (
                tc.tile_pool(name="agg_ps", bufs=2, space="PSUM") as agg_psp,
                tc.tile_pool(name="h1_ps", bufs=2, space="PSUM") as h1_psp,
            ):
              for wb in range(WB):
                w0 = 2 * wb
                msg = msgp.tile([128, 2 * TT * 128], MDT, tag="msg")
                nc.sync.dma_start(
                    msg[:, :],
                    msgs.ap()[:, w0 * TT * 128:(w0 + 2) * TT * 128])
                # one-hot S for overflow tiles of both windows
                S = sp.tile([128, 2 * NOV * 128], FP8 if MSG_FP8 else BF16,
                            tag="S")
                nc.vector.tensor_tensor(
                    out=S[:, :].rearrange("p (t j) -> p t j", j=128),
                    in0=iota_sb[:, :].rearrange("p (x j) -> p x j", x=1)
                        .to_broadcast([128, 2 * NOV, 128]),
                    in1=sel_sb[:, w0 * NOV:(w0 + 2) * NOV]
                        .to_broadcast([128, 2 * NOV, 128]),
                    op=OP.is_equal,
                )
                # agg psum layout: [w0_n | w1_n | w0_d | w1_d] (128 each).
                # One accumulation chain for the whole bank: first matmul
                # start=True, last stop=True; each byte is lazily zeroed on
                # its first write after start.
                agg = agg_psp.tile([128, 512], F32, tag="agg")
                first = True
                for i in range(2):
                    mbase = i * TT * 128
                    for ty in range(2):
                        dst_sl = slice((2 * ty + i) * 128,
                                       (2 * ty + i + 1) * 128)
                        tbase = mbase + ty * TPT * 128
                        last_grp = (i == 1 and ty == 1)
                        # self-edge: fold +x into the aggregate via a
                        # transpose-matmul of the resident node-major x
                        nc.tensor.matmul(
                            agg[:, dst_sl],
                            lhsT=xself_sb[:, (w0 + i) * 128:
                                          (w0 + i + 1) * 128],
                            rhs=wslice(K_I),
                            start=first, stop=False)
                        first = False
                        if MSG_FP8:
                            # DoubleRow: two 128-slot tiles per matmul
                            for t in range(0, KI - 1, 2):
                                a = tbase + t * 128
                                nc.tensor.matmul(
                                    agg[:, dst_sl],
                                    lhsT=msg[:, a:a + 256].rearrange(
                                        "p (t j) -> p t j", t=2),
                                    rhs=ipair_sb[:, :].rearrange(
                                        "p (t j) -> p t j", t=2),
                                    perf_mode=PM.DoubleRow,
                                    start=first, stop=False)
                                first = False
                            if KI % 2:
                                a = tbase + (KI - 1) * 128
                                nc.tensor.matmul(
                                    agg[:, dst_sl], lhsT=msg[:, a:a + 128],
                                    rhs=ipair_sb[:, 0:128],
                                    start=first, stop=False)
                                first = False
                            sbase = (i * 2 + ty) * OV * 128
                            for t in range(0, OV - 1, 2):
                                a = tbase + (KI + t) * 128
                                s = sbase + t * 128
                                nc.tensor.matmul(
                                    agg[:, dst_sl],
                                    lhsT=msg[:, a:a + 256].rearrange(
                                        "p (t j) -> p t j", t=2),
                                    rhs=S[:, s:s + 256].rearrange(
                                        "p (t j) -> p t j", t=2),
                                    perf_mode=PM.DoubleRow,
                                    start=False,
                                    stop=(last_grp and t == OV - 2))
                            if OV % 2:
                                a = tbase + (KI + OV - 1) * 128
                                s = sbase + (OV - 1) * 128
                                nc.tensor.matmul(
                                    agg[:, dst_sl], lhsT=msg[:, a:a + 128],
                                    rhs=S[:, s:s + 128],
                                    start=False, stop=last_grp)
                        else:
                            for t in range(KI):
                                nc.tensor.matmul(
                                    agg[:, dst_sl],
                                    lhsT=msg[:, tbase + t * 128:
                                             tbase + (t + 1) * 128],
                                    rhs=wslice(K_I),
                                    start=first, stop=False)
                                first = False
                            for t in range(OV):
                                scol = ((i * 2 + ty) * OV + t) * 128
                                nc.tensor.matmul(
                                    agg[:, dst_sl],
                                    lhsT=msg[:, tbase + (KI + t) * 128:
                                             tbase + (KI + t + 1) * 128],
                                    rhs=S[:, scol:scol + 128],
                                    start=False,
                                    stop=(last_grp and t == OV - 1))
                # hx = agg (self-edges already added x); psum -> sbuf bf16
                hx = hxp.tile([128, 512], BF16, tag="hx")
                nc.scalar.activation(hx[:, :], agg[:, :], ACT.Identity)
                h1 = h1_psp.tile([128, 512], F32, tag="h1")
                nc.tensor.matmul(h1[:, 0:256], lhsT=wslice(K_W1N),
                                 rhs=hx[:, 0:256], start=True, stop=False)
                nc.tensor.matmul(h1[:, 256:512], lhsT=wslice(K_W1D),
                                 rhs=hx[:, 256:512], start=False, stop=True)
                # copy psum -> resident bf16 (no bias!) + row-sums on ACT
                nsl = slice(w0 * 128, (w0 + 2) * 128)
                nc.scalar.activation(
                    h1n_sb[:, nsl], h1[:, 0:256], ACT.Identity,
                    accum_out=stat_s[:, 2 * wb:2 * wb + 1])
                nc.scalar.activation(
                    h1d_sb[:, nsl], h1[:, 256:512], ACT.Identity,
                    accum_out=stat_s[:, 2 * wb + 1:2 * wb + 2])
                # sum of squares from the bf16 copies on DVE (2x mode)
                sq = sqp.tile([128, 512], BF16, tag="sq")
                if USE_STT:
                    nc.vector.scalar_tensor_tensor(
                        out=sq[:, 0:256], in0=h1n_sb[:, nsl], scalar=1.0,
                        in1=h1n_sb[:, nsl], op0=OP.mult, op1=OP.mult,
                        accum_out=stat_q[:, 2 * wb:2 * wb + 1])
                    nc.vector.scalar_tensor_tensor(
                        out=sq[:, 256:512], in0=h1d_sb[:, nsl], scalar=1.0,
                        in1=h1d_sb[:, nsl], op0=OP.mult, op1=OP.mult,
                        accum_out=stat_q[:, 2 * wb + 1:2 * wb + 2])
                elif USE_TTR:
                    nc.vector.tensor_tensor_reduce(
                        out=sq[:, 0:256], in0=h1n_sb[:, nsl],
                        in1=h1n_sb[:, nsl],
                        scale=1.0, scalar=0.0, op0=OP.mult, op1=OP.add,
                        accum_out=stat_q[:, 2 * wb:2 * wb + 1])
                    nc.vector.tensor_tensor_reduce(
                        out=sq[:, 256:512], in0=h1d_sb[:, nsl],
                        in1=h1d_sb[:, nsl],
                        scale=1.0, scalar=0.0, op0=OP.mult, op1=OP.add,
                        accum_out=stat_q[:, 2 * wb + 1:2 * wb + 2])
                else:
                    nc.vector.tensor_tensor(sq[:, 0:256], h1n_sb[:, nsl],
                                            h1n_sb[:, nsl], op=OP.mult)
                    nc.vector.tensor_tensor(sq[:, 256:512], h1d_sb[:, nsl],
                                            h1d_sb[:, nsl], op=OP.mult)
                    nc.vector.reduce_sum(
                        out=stat_q[:, 2 * wb:2 * wb + 1],
                        in_=sq[:, 0:256], axis=AX.X)
                    nc.vector.reduce_sum(
                        out=stat_q[:, 2 * wb + 1:2 * wb + 2],
                        in_=sq[:, 256:512], axis=AX.X)

            # ---------- stats reduce + allreduce + BN params ----------
            sums = smallp.tile([128, 4], F32, tag="sums")
            # col order: [sum_n, sumsq_n, sum_d, sumsq_d]
            for br in range(2):
                nc.vector.reduce_sum(
                    out=sums[:, 2 * br:2 * br + 1],
                    in_=stat_s[:, :].rearrange("p (w k) -> p w k", k=2)
                        [:, :, br],
                    axis=AX.X)
                nc.vector.reduce_sum(
                    out=sums[:, 2 * br + 1:2 * br + 2],
                    in_=stat_q[:, :].rearrange("p (w k) -> p w k", k=2)
                        [:, :, br],
                    axis=AX.X)
            cc_in = dram.tile([128, 4], F32)
            cc_out = dram.tile([128, 4], F32)
            nc.gpsimd.dma_start(cc_in[:], sums[:])
            nc.gpsimd.collective_compute(
                "AllReduce", OP.add,
                replica_groups=[list(range(cfg.C))],
                ins=[cc_in[:].opt()], outs=[cc_out[:].opt()],
            )
            gsums = smallp.tile([128, 4], F32, tag="gsums")
            nc.gpsimd.dma_start(gsums[:], cc_out[:])

            inv_n = 1.0 / cfg.N
            for br, (b1c, g_col, b_col) in enumerate([
                (V_B1N, V_GN, V_BN),
                (V_B1D, V_GD, V_BD),
            ]):
                mean = smallp.tile([128, 1], F32, tag="mean")
                nc.vector.tensor_scalar(
                    out=mean[:], in0=gsums[:, 2 * br:2 * br + 1],
                    scalar1=inv_n, scalar2=None, op0=OP.mult)
                ex2 = smallp.tile([128, 1], F32, tag="ex2")
                nc.vector.tensor_scalar(
                    out=ex2[:], in0=gsums[:, 2 * br + 1:2 * br + 2],
                    scalar1=inv_n, scalar2=None, op0=OP.mult)
                var = smallp.tile([128, 1], F32, tag="var")
                nc.vector.tensor_tensor(var[:], mean[:], mean[:], op=OP.mult)
                nc.vector.tensor_tensor(var[:], ex2[:], var[:],
                                        op=OP.subtract)
                # rstd = 1/sqrt(var + eps); scale = gamma * rstd  (> 0)
                nc.vector.tensor_scalar(out=var[:], in0=var[:],
                                        scalar1=BN_EPS, scalar2=None,
                                        op0=OP.add)
                std = smallp.tile([128, 1], F32, tag="std")
                nc.scalar.activation(std[:], var[:], ACT.Sqrt)
                rstd = smallp.tile([128, 1], F32, tag="rstd")
                nc.vector.reciprocal(rstd[:], std[:])
                sc = smallp.tile([128, 1], F32, tag="sc")
                nc.vector.tensor_tensor(sc[:], vcol(g_col), rstd[:],
                                        op=OP.mult)
                # c = beta / scale - mean  (the w1 bias cancels inside BN)
                rsc = smallp.tile([128, 1], F32, tag="rsc")
                nc.vector.reciprocal(rsc[:], sc[:])
                bos = smallp.tile([128, 1], F32, tag="bos")
                nc.vector.tensor_tensor(bos[:], vcol(b_col), rsc[:],
                                        op=OP.mult)
                nc.vector.tensor_tensor(cvec[:, br:br + 1], bos[:], mean[:],
                                        op=OP.subtract)
                # fold scale into second-layer weights
                wsl2 = wslice(K_W2N) if br == 0 else wslice(K_W2DF)
                wdst = w2n_s if br == 0 else w2df_s
                nc.vector.tensor_scalar(out=wdst[:], in0=wsl2,
                                        scalar1=sc[:], scalar2=None,
                                        op0=OP.mult)
                msl = wslice(K_M1T) if br == 0 else wslice(K_M2T)
                mdst = m1s if br == 0 else m2s
                nc.vector.tensor_scalar(out=mdst[:], in0=msl,
                                        scalar1=sc[:], scalar2=None,
                                        op0=OP.mult)

            # ---------- phase C: BN/relu, gate, outputs ----------
            with (
                tc.tile_pool(name="z_ps", bufs=2, space="PSUM") as z_psp,
                tc.tile_pool(name="nm_ps", bufs=3, space="PSUM") as nm_psp,
            ):
              for wb in range(WB):
                w0 = 2 * wb
                nsl = slice(w0 * 128, (w0 + 2) * 128)
                hbn = hbnp.tile([128, 512], BF16, tag="hbn")
                heng = nc.gpsimd if USE_POOL_HBN else nc.vector
                heng.tensor_scalar(
                    out=hbn[:, 0:256], in0=h1n_sb[:, nsl],
                    scalar1=cvec[:, 0:1], scalar2=0.0,
                    op0=OP.add, op1=OP.max)
                heng.tensor_scalar(
                    out=hbn[:, 256:512], in0=h1d_sb[:, nsl],
                    scalar1=cvec[:, 1:2], scalar2=0.0,
                    op0=OP.add, op1=OP.max)
                # gate logits via composed weights (bank-padded psum tile)
                z = z_psp.tile([128, 512], F32, tag="z")
                nc.tensor.matmul(z[:, 0:256], lhsT=wslice(K_M0T),
                                 rhs=xT_sb[:, nsl], start=True, stop=False)
                nc.tensor.matmul(z[:, 0:256], lhsT=m1s[:], rhs=hbn[:, 0:256],
                                 start=False, stop=False)
                nc.tensor.matmul(z[:, 0:256], lhsT=m2s[:],
                                 rhs=hbn[:, 256:512],
                                 start=False, stop=True)
                e = ep.tile([128, 256], BF16, tag="e")
                nc.scalar.activation(e[:], z[:, 0:256], ACT.Exp,
                                     bias=vcol(V_BZ))
                # nm: one psum bank per window: [ct | At | xd | pad], one
                # accumulation chain per bank.
                nm = nm_psp.tile([128, 1024], F32, tag="nm")
                for i in range(2):
                    b = i * 512
                    isl = slice((w0 + i) * 128, (w0 + i + 1) * 128)
                    nc.tensor.matmul(nm[:, b:b + 128],
                                     lhsT=e[:, i * 128:(i + 1) * 128],
                                     rhs=wslice(K_U), start=True, stop=False)
                    nc.tensor.matmul(nm[:, b + 128:b + 256],
                                     lhsT=xT_sb[:, isl], rhs=wslice(K_WSL),
                                     start=False, stop=False)
                    nc.tensor.matmul(nm[:, b + 128:b + 256],
                                     lhsT=hbn[:, i * 128:(i + 1) * 128],
                                     rhs=w2n_s[:], start=False, stop=False)
                    nc.tensor.matmul(nm[:, b + 256:b + 384],
                                     lhsT=hbn[:, 256 + i * 128:
                                              256 + (i + 1) * 128],
                                     rhs=w2df_s[:], start=False, stop=False)
                    # bias rank-1 spanning At|xd, closes the chain
                    nc.tensor.matmul(nm[:, b + 128:b + 384],
                                     lhsT=ones_sb[:], rhs=rows_sb[:, 0:256],
                                     start=False, stop=True)
                nmv = nm[:, :].rearrange("p (i q) -> p i q", q=512)
                r = smallp.tile([128, 2], F32, tag="r")
                nc.vector.reciprocal(
                    r[:, :].rearrange("p (i u) -> p i u", u=1),
                    nmv[:, :, 127:128])
                t1 = t1p.tile([128, 256], BF16, tag="t1")
                for i in range(2):
                    if USE_ACT_T1:
                        nc.scalar.activation(
                            t1[:, i * 128:(i + 1) * 128],
                            nm[:, i * 512:i * 512 + 128],
                            ACT.Identity, scale=r[:, i:i + 1])
                    else:
                        nc.vector.tensor_scalar(
                            out=t1[:, i * 128:(i + 1) * 128],
                            in0=nm[:, i * 512:i * 512 + 128],
                            scalar1=r[:, i:i + 1], scalar2=None,
                            op0=OP.mult)
                u = up.tile([128, 256], BF16, tag="u")
                nc.vector.tensor_tensor(
                    out=u[:, :].rearrange("p (i q) -> p i q", i=2),
                    in0=t1[:, :].rearrange("p (i q) -> p i q", i=2),
                    in1=nmv[:, :, 256:384], op=OP.mult)
                o = outp.tile([128, 256], F32, tag="o")
                nc.vector.tensor_tensor(
                    out=o[:, :].rearrange("p (i q) -> p i q", i=2),
                    in0=u[:, :].rearrange("p (i q) -> p i q", i=2),
                    in1=nmv[:, :, 128:256], op=OP.add)
                nc.sync.dma_start(
                    out.ap()[w0 * 128:(w0 + 2) * 128, :]
                       .rearrange("(i p) f -> p i f", i=2),
                    o[:, :].rearrange("p (i f) -> p i f", i=2))

    nc.compile()
    return nc


def pack_edges(cfg: Cfg, src, dst, et):
    """Slot assignment. Returns (off [128, W*TT] int32 per core list,
    sel [128, W*NOV] f32 per core list). Raises if OV capacity exceeded."""
    C, W, KI, OV, TPT, TT, npc = (cfg.C, cfg.W, cfg.KI, cfg.OV, cfg.TPT,
                                  cfg.TT, cfg.npc)
    NOV = 2 * OV
    E = src.shape[0]
    core = dst // npc
    ldst = dst - core * npc
    wdw = ldst >> 7
    j = ldst & 127

    # stable sort by (core, window, type, j)
    gkey = ((core.astype(np.int64) * W + wdw) * 2 + et)
    fkey = gkey * 128 + j
    order = np.argsort(fkey, kind="stable")
    fs = fkey[order]
    gs = gkey[order]
    js = j[order]
    srcs = src[order]

    # rank within (c,w,t,j)
    fcounts = np.bincount(fs, minlength=cfg.C * W * 2 * 128)
    fstarts = np.concatenate([[0], np.cumsum(fcounts)[:-1]])
    rank = np.arange(E, dtype=np.int64) - fstarts[fs]

    id_mask = rank < KI
    ov_mask = ~id_mask
    # overflow rank within (c,w,t)
    cum = np.cumsum(ov_mask)
    gcounts = np.bincount(gs, minlength=cfg.C * W * 2)
    gstarts = np.concatenate([[0], np.cumsum(gcounts)[:-1]])
    cum_at_start = np.where(gstarts > 0, cum[gstarts - 1], 0)
    ovr = cum - 1 - cum_at_start[gs]

    max_ov = int((ovr[ov_mask].max() + 1) if ov_mask.any() else 0)
    if max_ov > OV * 128:
        raise RuntimeError(f"overflow capacity exceeded: {max_ov} > {OV*128}")

    cores_s = (gs // (2 * W)).astype(np.int64)
    w_s = (gs // 2) % W
    ty_s = gs % 2

    # tile index within window and partition
    tile_idx = np.where(id_mask, rank, KI + (ovr >> 7))
    part = np.where(id_mask, js, ovr & 127)
    col = w_s * TT + ty_s * TPT + tile_idx

    ZROW = cfg.N
    off = np.full((C, 128, W * TT), ZROW, np.int32)
    off[cores_s, part, col] = srcs

    sel = np.full((C, 128, W * NOV), -1.0, np.float32)
    ov_idx = np.nonzero(ov_mask)[0]
    scol = (w_s[ov_idx] * NOV + ty_s[ov_idx] * OV
            + (ovr[ov_idx] >> 7))
    sel[cores_s[ov_idx], ovr[ov_idx] & 127, scol] = js[ov_idx].astype(
        np.float32)
    return off, sel


def prep_inputs(cfg: Cfg, x, edge_index, edge_type, w_sl, b_sl,
                w1_n, b1_n, gamma_n, beta_n, w2_n, b2_n,
                w1_d, b1_d, gamma_d, beta_d, w2_d, b2_d,
                w_gat, b_gat):
    C, npc, npad = cfg.C, cfg.npc, cfg.npad
    x = np.asarray(x, np.float32)
    src = np.asarray(edge_index[0], np.int64).astype(np.int64)
    dst = np.asarray(edge_index[1], np.int64).astype(np.int64)
    et = np.asarray(edge_type, np.int64).astype(np.int64)

    off, sel = pack_edges(cfg, src, dst, et)

    xbf = np.vstack([x, np.zeros((1, 128), np.float32)]).astype(
        F8 if MSG_FP8 else BF)
    msgs = [np.ascontiguousarray(xbf[off[c]].reshape(128, -1))
            for c in range(C)]

    xTs = []
    xselfs = []
    W = cfg.W
    for c in range(C):
        xp = np.zeros((npad, 128), np.float32)
        xp[:npc] = x[c * npc:(c + 1) * npc]
        xTs.append(np.ascontiguousarray(xp.T).astype(BF))
        xselfs.append(np.ascontiguousarray(
            xp.reshape(W, 128, 128).transpose(1, 0, 2)
              .reshape(128, npad)).astype(BF))

    def bt(a):
        return np.ascontiguousarray(np.asarray(a, np.float64)).astype(BF)

    w_sl64 = np.asarray(w_sl, np.float64)
    w2n64 = np.asarray(w2_n, np.float64)
    w2d64 = np.asarray(w2_d, np.float64)
    wg = np.asarray(w_gat, np.float64)
    wg0, wg1, wg2 = wg[:, 0:128], wg[:, 128:256], wg[:, 256:384]

    wcols = [
        bt(w_sl64.T), bt(np.asarray(w1_n).T), bt(np.asarray(w1_d).T),
        bt(w2n64.T), bt(w2d64[::-1, :].T),
        bt((wg0 @ w_sl64).T), bt((wg1 @ w2n64).T), bt((wg2 @ w2d64).T),
        bt(np.triu(np.ones((128, 128), np.float32))),
        bt(np.eye(128, dtype=np.float32)),
    ]
    wpack = np.concatenate(wcols, axis=1)

    rows = np.concatenate([
        (np.asarray(b_sl, np.float64) + np.asarray(b2_n, np.float64))[None, :],
        np.asarray(b2_d, np.float64)[::-1][None, :],
    ], axis=1).astype(BF)

    bz = (np.asarray(b_gat, np.float64) + wg0 @ np.asarray(b_sl, np.float64)
          + wg1 @ np.asarray(b2_n, np.float64)
          + wg2 @ np.asarray(b2_d, np.float64))
    vecs = np.stack([
        np.asarray(b1_n, np.float64), np.asarray(b1_d, np.float64),
        np.asarray(gamma_n, np.float64), np.asarray(beta_n, np.float64),
        np.asarray(gamma_d, np.float64), np.asarray(beta_d, np.float64),
        bz,
    ], axis=1).astype(np.float32)

    in_maps = []
    for c in range(C):
        m = {
            "msgs": msgs[c],
            "xT": xTs[c],
            "xself": xselfs[c],
            "sel": np.ascontiguousarray(sel[c]),
            "wpack": wpack,
            "rows": rows,
            "vecs": vecs,
            "iota128": np.broadcast_to(
                np.arange(128, dtype=np.float32)[None, :],
                (128, 128)).copy(),
        }
        if MSG_FP8:
            m["ipair"] = np.concatenate(
                [np.eye(128, dtype=np.float32)] * 2, axis=1).astype(F8)
        in_maps.append(m)
    return in_maps


_BUILD_CACHE = {}


def run(cfg: Cfg, inputs: dict, **run_kwargs):
    in_maps = None
    while True:
        try:
            in_maps = prep_inputs(cfg, **inputs)
            break
        except RuntimeError:
            cfg = Cfg(cfg.N, cfg.E, cfg.C, cfg.KI, cfg.OV + 1)
    key = (cfg.N, cfg.E, cfg.C, cfg.KI, cfg.OV,
           USE_TTR, USE_STT, USE_POOL_HBN, USE_ACT_T1, MSG_FP8)
    if key not in _BUILD_CACHE:
        _BUILD_CACHE[key] = build(cfg)
    nc = _BUILD_CACHE[key]
    res = run_bass_kernel_spmd(nc, in_maps, core_ids=list(range(cfg.C)),
                               **run_kwargs)
    outs = [res.results[c]["out"][:cfg.npc] for c in range(cfg.C)]
    return np.concatenate(outs, axis=0).astype(np.float32), res


def kernel(**inputs):
    out, _ = run(CFG, inputs)
    return out


# revision 48
# speedup vs baseline: 2.0581x; 1.1199x over previous
"""Trainium2 Bass kernel for a 2-relation GIN-style GNN message-passing layer.

Full (unsharded) inputs in, full output out. Internally:
  - nodes sharded across 8 NeuronCores (12500/core, padded to 12544 = 98
    windows of 128); edges partitioned by destination-node shard (CPU prep).
  - per (window, relation), edges are packed into fixed tiles of 128 slots:
      * KI "identity" tiles: the t-th edge of destination j sits in
        partition j of tile t, so segment-sum over a tile is a plain
        transpose-accumulate: matmul(lhsT=msg_tile, rhs=I128). Empty slots
        gather a zero row of x.
      * OV "overflow" one-hot tiles for edges beyond KI per destination:
        matmul with a one-hot scatter matrix S built on-device via is_equal
        (padding slots sel=-1 give zero columns).
  - per-edge source rows are pre-gathered on CPU into a bf16 stream so the
    device sees only contiguous DMA.
  - BatchNorm batch stats are computed bias-free (bias folded analytically
    into the post-BN shift), via fused copy+row-sum (scalar engine
    accumulate) and fused square+reduce (DVE tensor_tensor_reduce), and
    AllReduce'd across the 8 cores in-kernel.
  - the BN scale is folded into the second-layer weights at runtime
    (requires gamma > 0, true for this model), so BN+ReLU is a single
    add+max op.
  - gate logits are computed with CPU-composed weights
    (w_gat_chunk @ w_branch), skipping the feature-major x_new_* tensors
    entirely; cumsum = matmul with triangular ones; flip folded into
    reversed weight rows; node-major outputs via data-stationary matmuls.
"""

import numpy as np
import ml_dtypes

import concourse.bass as bass
import concourse.mybir as mybir
import concourse.tile as tile
from concourse import bacc
from concourse.bass_utils import run_bass_kernel_spmd

F32 = mybir.dt.float32
BF16 = mybir.dt.bfloat16
FP8 = mybir.dt.float8e4
AX = mybir.AxisListType
OP = mybir.AluOpType
ACT = mybir.ActivationFunctionType
PM = mybir.MatmulPerfMode

BF = ml_dtypes.bfloat16
F8 = ml_dtypes.float8_e4m3


class Cfg:
    def __init__(self, N, E, C, KI, OV):
        self.N = N            # total nodes
        self.E = E            # total edges
        self.C = C            # cores
        self.F = 128
        self.KI = KI          # identity tiles per (window, type)
        self.OV = OV          # one-hot overflow tiles per (window, type)
        self.TPT = KI + OV    # tiles per type
        self.TT = 2 * self.TPT  # tiles per window (both types)
        assert N % C == 0
        self.npc = N // C                      # real nodes per core
        self.W = (self.npc + 127) // 128       # windows per core
        assert self.W % 2 == 0
        self.WB = self.W // 2                  # 2-window iterations
        self.npad = self.W * 128               # padded nodes per core


CFG = Cfg(N=100000, E=1600000, C=8, KI=8, OV=2)

# column layout of the "vecs" [128, 7] f32 input
(V_B1N, V_B1D, V_GN, V_BN, V_GD, V_BD, V_BZ) = range(7)

# column layout of wpack [128, 128*10] bf16
(K_WSL, K_W1N, K_W1D, K_W2N, K_W2DF, K_M0T, K_M1T, K_M2T, K_U, K_I) = range(10)

BN_EPS = 1e-5


USE_TTR = False       # tensor_tensor_reduce hangs TRN2 HW via this path
USE_STT = True        # fused square+accumulate via scalar_tensor_tensor
USE_POOL_HBN = False  # gpsimd tensor ops are ~8x slower than modeled
USE_ACT_T1 = True     # t1 via ACT Identity+scale (else DVE tensor_scalar)
MSG_FP8 = True        # fp8e4m3 message stream + DoubleRow paired matmuls


def build(cfg: Cfg):
    nc = bacc.Bacc("TRN2", target_bir_lowering=False, debug=False,
                   num_devices=cfg.C)
    W, WB, KI, OV, TPT, TT, npad = (cfg.W, cfg.WB, cfg.KI, cfg.OV,
                                    cfg.TPT, cfg.TT, cfg.npad)
    NOV = 2 * OV   # overflow tiles per window (both types)

    MDT = FP8 if MSG_FP8 else BF16
    msgs = nc.dram_tensor("msgs", [128, W * TT * 128], MDT,
                          kind="ExternalInput")
    xself = nc.dram_tensor("xself", [128, npad], BF16, kind="ExternalInput")
    if MSG_FP8:
        ipair = nc.dram_tensor("ipair", [128, 256], FP8,
                               kind="ExternalInput")
    xT = nc.dram_tensor("xT", [128, npad], BF16, kind="ExternalInput")
    sel = nc.dram_tensor("sel", [128, W * NOV], F32, kind="ExternalInput")
    wpack = nc.dram_tensor("wpack", [128, 128 * 10], BF16, kind="ExternalInput")
    rows = nc.dram_tensor("rows", [1, 256], BF16, kind="ExternalInput")
    vecs = nc.dram_tensor("vecs", [128, 7], F32, kind="ExternalInput")
    iota_in = nc.dram_tensor("iota128", [128, 128], F32, kind="ExternalInput")
    out = nc.dram_tensor("out", [npad, 128], F32, kind="ExternalOutput")

    with tile.TileContext(nc) as tc:
        with (
            tc.tile_pool(name="res", bufs=1) as res,
            tc.tile_pool(name="msgp", bufs=3) as msgp,
            tc.tile_pool(name="sp", bufs=3) as sp,
            tc.tile_pool(name="hxp", bufs=3) as hxp,
            tc.tile_pool(name="sqp", bufs=3) as sqp,
            tc.tile_pool(name="smallp", bufs=8) as smallp,
            tc.tile_pool(name="dram", bufs=1, space="DRAM") as dram,
            tc.tile_pool(name="hbnp", bufs=3) as hbnp,
            tc.tile_pool(name="ep", bufs=3) as ep,
            tc.tile_pool(name="t1p", bufs=3) as t1p,
            tc.tile_pool(name="up", bufs=3) as up,
            tc.tile_pool(name="outp", bufs=3) as outp,
        ):
            # ---------- resident loads ----------
            xT_sb = res.tile([128, npad], BF16)
            nc.sync.dma_start(xT_sb[:], xT.ap())
            xself_sb = res.tile([128, npad], BF16)
            nc.sync.dma_start(xself_sb[:], xself.ap())
            sel_sb = res.tile([128, W * NOV], F32)
            nc.sync.dma_start(sel_sb[:], sel.ap())
            wp = res.tile([128, 128 * 10], BF16)
            nc.sync.dma_start(wp[:], wpack.ap())
            rows_sb = res.tile([1, 256], BF16)
            nc.sync.dma_start(rows_sb[:], rows.ap())
            vec = res.tile([128, 7], F32)
            nc.sync.dma_start(vec[:], vecs.ap())
            iota_sb = res.tile([128, 128], F32)
            nc.sync.dma_start(iota_sb[:], iota_in.ap())
            if MSG_FP8:
                ipair_sb = res.tile([128, 256], FP8)
                nc.sync.dma_start(ipair_sb[:], ipair.ap())

            h1n_sb = res.tile([128, npad], BF16)
            h1d_sb = res.tile([128, npad], BF16)
            ones_sb = res.tile([1, 128], BF16)
            nc.vector.memset(ones_sb[:], 1.0)
            stat_s = res.tile([128, 2 * WB], F32)   # sums (ACT accum)
            stat_q = res.tile([128, 2 * WB], F32)   # sumsq (DVE accum)
            # runtime BN-folded params
            cvec = res.tile([128, 2], F32)          # relu shift per branch
            w2n_s = res.tile([128, 128], BF16)
            w2df_s = res.tile([128, 128], BF16)
            m1s = res.tile([128, 128], BF16)
            m2s = res.tile([128, 128], BF16)

            def wslice(k):
                return wp[:, k * 128:(k + 1) * 128]

            def vcol(k):
                return vec[:, k:k + 1]

            # ---------- phase A: aggregate + first linear + stats ----------
            with (
                tc.tile_pool(name="agg_ps", bufs=2, space="PSUM") as agg_psp,
                tc.tile_pool(name="h1_ps", bufs=2, space="PSUM") as h1_psp,
            ):
              for wb in range(WB):
                w0 = 2 * wb
                msg = msgp.tile([128, 2 * TT * 128], MDT, tag="msg")
                nc.sync.dma_start(
                    msg[:, :],
                    msgs.ap()[:, w0 * TT * 128:(w0 + 2) * TT * 128])
                # one-hot S for overflow tiles of both windows
                S = sp.tile([128, 2 * NOV * 128], FP8 if MSG_FP8 else BF16,
                            tag="S")
                nc.vector.tensor_tensor(
                    out=S[:, :].rearrange("p (t j) -> p t j", j=128),
                    in0=iota_sb[:, :].rearrange("p (x j) -> p x j", x=1)
                        .to_broadcast([128, 2 * NOV, 128]),
                    in1=sel_sb[:, w0 * NOV:(w0 + 2) * NOV]
                        .to_broadcast([128, 2 * NOV, 128]),
                    op=OP.is_equal,
                )
                # agg psum layout: [w0_n | w1_n | w0_d | w1_d] (128 each).
                # One accumulation chain for the whole bank: first matmul
                # start=True, last stop=True; each byte is lazily zeroed on
                # its first write after start.
                agg = agg_psp.tile([128, 512], F32, tag="agg")
                first = True
                for i in range(2):
                    mbase = i * TT * 128
                    for ty in range(2):
                        dst_sl = slice((2 * ty + i) * 128,
                                       (2 * ty + i + 1) * 128)
                        tbase = mbase + ty * TPT * 128
                        last_grp = (i == 1 and ty == 1)
                        # self-edge: fold +x into the aggregate via a
                        # transpose-matmul of the resident node-major x
                        nc.tensor.matmul(
                            agg[:, dst_sl],
                            lhsT=xself_sb[:, (w0 + i) * 128:
                                          (w0 + i + 1) * 128],
                            rhs=wslice(K_I),
                            start=first, stop=False)
                        first = False
                        if MSG_FP8:
                            # DoubleRow: two 128-slot tiles per matmul
                            for t in range(0, KI - 1, 2):
                                a = tbase + t * 128
                                nc.tensor.matmul(
                                    agg[:, dst_sl],
                                    lhsT=msg[:, a:a + 256].rearrange(
                                        "p (t j) -> p t j", t=2),
                                    rhs=ipair_sb[:, :].rearrange(
                                        "p (t j) -> p t j", t=2),
                                    perf_mode=PM.DoubleRow,
                                    start=first, stop=False)
                                first = False
                            if KI % 2:
                                a = tbase + (KI - 1) * 128
                                nc.tensor.matmul(
                                    agg[:, dst_sl], lhsT=msg[:, a:a + 128],
                                    rhs=ipair_sb[:, 0:128],
                                    start=first, stop=False)
                                first = False
                            sbase = (i * 2 + ty) * OV * 128
                            for t in range(0, OV - 1, 2):
                                a = tbase + (KI + t) * 128
                                s = sbase + t * 128
                                nc.tensor.matmul(
                                    agg[:, dst_sl],
                                    lhsT=msg[:, a:a + 256].rearrange(
                                        "p (t j) -> p t j", t=2),
                                    rhs=S[:, s:s + 256].rearrange(
                                        "p (t j) -> p t j", t=2),
                                    perf_mode=PM.DoubleRow,
                                    start=False,
                                    stop=(last_grp and t == OV - 2))
                            if OV % 2:
                                a = tbase + (KI + OV - 1) * 128
                                s = sbase + (OV - 1) * 128
                                nc.tensor.matmul(
                                    agg[:, dst_sl], lhsT=msg[:, a:a + 128],
                                    rhs=S[:, s:s + 128],
                                    start=False, stop=last_grp)
                        else:
                            for t in range(KI):
                                nc.tensor.matmul(
                                    agg[:, dst_sl],
                                    lhsT=msg[:, tbase + t * 128:
                                             tbase + (t + 1) * 128],
                                    rhs=wslice(K_I),
                                    start=first, stop=False)
                                first = False
                            for t in range(OV):
                                scol = ((i * 2 + ty) * OV + t) * 128
                                nc.tensor.matmul(
                                    agg[:, dst_sl],
                                    lhsT=msg[:, tbase + (KI + t) * 128:
                                             tbase + (KI + t + 1) * 128],
                                    rhs=S[:, scol:scol + 128],
                                    start=False,
                                    stop=(last_grp and t == OV - 1))
                # hx = agg (self-edges already added x); psum -> sbuf bf16
                hx = hxp.tile([128, 512], BF16, tag="hx")
                nc.vector.tensor_scalar(out=hx[:, :], in0=agg[:, :],
                                        scalar1=0.0, scalar2=None,
                                        op0=OP.add)
                h1 = h1_psp.tile([128, 512], F32, tag="h1")
                nc.tensor.matmul(h1[:, 0:256], lhsT=wslice(K_W1N),
                                 rhs=hx[:, 0:256], start=True, stop=False)
                nc.tensor.matmul(h1[:, 256:512], lhsT=wslice(K_W1D),
                                 rhs=hx[:, 256:512], start=False, stop=True)
                # copy psum -> resident bf16 (no bias!) + row-sums on ACT
                nsl = slice(w0 * 128, (w0 + 2) * 128)
                nc.scalar.activation(
                    h1n_sb[:, nsl], h1[:, 0:256], ACT.Identity,
                    accum_out=stat_s[:, 2 * wb:2 * wb + 1])
                nc.scalar.activation(
                    h1d_sb[:, nsl], h1[:, 256:512], ACT.Identity,
                    accum_out=stat_s[:, 2 * wb + 1:2 * wb + 2])
                # sum of squares from the bf16 copies on DVE (2x mode)
                sq = sqp.tile([128, 512], BF16, tag="sq")
                if USE_STT:
                    nc.vector.scalar_tensor_tensor(
                        out=sq[:, 0:256], in0=h1n_sb[:, nsl], scalar=1.0,
                        in1=h1n_sb[:, nsl], op0=OP.mult, op1=OP.mult,
                        accum_out=stat_q[:, 2 * wb:2 * wb + 1])
                    nc.vector.scalar_tensor_tensor(
                        out=sq[:, 256:512], in0=h1d_sb[:, nsl], scalar=1.0,
                        in1=h1d_sb[:, nsl], op0=OP.mult, op1=OP.mult,
                        accum_out=stat_q[:, 2 * wb + 1:2 * wb + 2])
                elif USE_TTR:
                    nc.vector.tensor_tensor_reduce(
                        out=sq[:, 0:256], in0=h1n_sb[:, nsl],
                        in1=h1n_sb[:, nsl],
                        scale=1.0, scalar=0.0, op0=OP.mult, op1=OP.add,
                        accum_out=stat_q[:, 2 * wb:2 * wb + 1])
                    nc.vector.tensor_tensor_reduce(
                        out=sq[:, 256:512], in0=h1d_sb[:, nsl],
                        in1=h1d_sb[:, nsl],
                        scale=1.0, scalar=0.0, op0=OP.mult, op1=OP.add,
                        accum_out=stat_q[:, 2 * wb + 1:2 * wb + 2])
                else:
                    nc.vector.tensor_tensor(sq[:, 0:256], h1n_sb[:, nsl],
                                            h1n_sb[:, nsl], op=OP.mult)
                    nc.vector.tensor_tensor(sq[:, 256:512], h1d_sb[:, nsl],
                                            h1d_sb[:, nsl], op=OP.mult)
                    nc.vector.reduce_sum(
                        out=stat_q[:, 2 * wb:2 * wb + 1],
                        in_=sq[:, 0:256], axis=AX.X)
                    nc.vector.reduce_sum(
                        out=stat_q[:, 2 * wb + 1:2 * wb + 2],
                        in_=sq[:, 256:512], axis=AX.X)

            # ---------- stats reduce + allreduce + BN params ----------
            sums = smallp.tile([128, 4], F32, tag="sums")
            # col order: [sum_n, sumsq_n, sum_d, sumsq_d]
            for br in range(2):
                nc.vector.reduce_sum(
                    out=sums[:, 2 * br:2 * br + 1],
                    in_=stat_s[:, :].rearrange("p (w k) -> p w k", k=2)
                        [:, :, br],
                    axis=AX.X)
                nc.vector.reduce_sum(
                    out=sums[:, 2 * br + 1:2 * br + 2],
                    in_=stat_q[:, :].rearrange("p (w k) -> p w k", k=2)
                        [:, :, br],
                    axis=AX.X)
            cc_in = dram.tile([128, 4], F32)
            cc_out = dram.tile([128, 4], F32)
            nc.gpsimd.dma_start(cc_in[:], sums[:])
            nc.gpsimd.collective_compute(
                "AllReduce", OP.add,
                replica_groups=[list(range(cfg.C))],
                ins=[cc_in[:].opt()], outs=[cc_out[:].opt()],
            )
            gsums = smallp.tile([128, 4], F32, tag="gsums")
            nc.gpsimd.dma_start(gsums[:], cc_out[:])

            inv_n = 1.0 / cfg.N
            for br, (b1c, g_col, b_col) in enumerate([
                (V_B1N, V_GN, V_BN),
                (V_B1D, V_GD, V_BD),
            ]):
                mean = smallp.tile([128, 1], F32, tag="mean")
                nc.vector.tensor_scalar(
                    out=mean[:], in0=gsums[:, 2 * br:2 * br + 1],
                    scalar1=inv_n, scalar2=None, op0=OP.mult)
                ex2 = smallp.tile([128, 1], F32, tag="ex2")
                nc.vector.tensor_scalar(
                    out=ex2[:], in0=gsums[:, 2 * br + 1:2 * br + 2],
                    scalar1=inv_n, scalar2=None, op0=OP.mult)
                var = smallp.tile([128, 1], F32, tag="var")
                nc.vector.tensor_tensor(var[:], mean[:], mean[:], op=OP.mult)
                nc.vector.tensor_tensor(var[:], ex2[:], var[:],
                                        op=OP.subtract)
                # rstd = 1/sqrt(var + eps); scale = gamma * rstd  (> 0)
                nc.vector.tensor_scalar(out=var[:], in0=var[:],
                                        scalar1=BN_EPS, scalar2=None,
                                        op0=OP.add)
                std = smallp.tile([128, 1], F32, tag="std")
                nc.scalar.activation(std[:], var[:], ACT.Sqrt)
                rstd = smallp.tile([128, 1], F32, tag="rstd")
                nc.vector.reciprocal(rstd[:], std[:])
                sc = smallp.tile([128, 1], F32, tag="sc")
                nc.vector.tensor_tensor(sc[:], vcol(g_col), rstd[:],
                                        op=OP.mult)
                # c = beta / scale - mean  (the w1 bias cancels inside BN)
                rsc = smallp.tile([128, 1], F32, tag="rsc")
                nc.vector.reciprocal(rsc[:], sc[:])
                bos = smallp.tile([128, 1], F32, tag="bos")
                nc.vector.tensor_tensor(bos[:], vcol(b_col), rsc[:],
                                        op=OP.mult)
                nc.vector.tensor_tensor(cvec[:, br:br + 1], bos[:], mean[:],
                                        op=OP.subtract)
                # fold scale into second-layer weights
                wsl2 = wslice(K_W2N) if br == 0 else wslice(K_W2DF)
                wdst = w2n_s if br == 0 else w2df_s
                nc.vector.tensor_scalar(out=wdst[:], in0=wsl2,
                                        scalar1=sc[:], scalar2=None,
                                        op0=OP.mult)
                msl = wslice(K_M1T) if br == 0 else wslice(K_M2T)
                mdst = m1s if br == 0 else m2s
                nc.vector.tensor_scalar(out=mdst[:], in0=msl,
                                        scalar1=sc[:], scalar2=None,
                                        op0=OP.mult)

            # ---------- phase C: BN/relu, gate, outputs ----------
            with (
                tc.tile_pool(name="z_ps", bufs=2, space="PSUM") as z_psp,
                tc.tile_pool(name="nm_ps", bufs=3, space="PSUM") as nm_psp,
            ):
              for wb in range(WB):
                w0 = 2 * wb
                nsl = slice(w0 * 128, (w0 + 2) * 128)
                hbn = hbnp.tile([128, 512], BF16, tag="hbn")
                heng = nc.gpsimd if USE_POOL_HBN else nc.vector
                heng.tensor_scalar(
                    out=hbn[:, 0:256], in0=h1n_sb[:, nsl],
                    scalar1=cvec[:, 0:1], scalar2=0.0,
                    op0=OP.add, op1=OP.max)
                heng.tensor_scalar(
                    out=hbn[:, 256:512], in0=h1d_sb[:, nsl],
                    scalar1=cvec[:, 1:2], scalar2=0.0,
                    op0=OP.add, op1=OP.max)
                # gate logits via composed weights (bank-padded psum tile)
                z = z_psp.tile([128, 512], F32, tag="z")
                nc.tensor.matmul(z[:, 0:256], lhsT=wslice(K_M0T),
                                 rhs=xT_sb[:, nsl], start=True, stop=False)
                nc.tensor.matmul(z[:, 0:256], lhsT=m1s[:], rhs=hbn[:, 0:256],
                                 start=False, stop=False)
                nc.tensor.matmul(z[:, 0:256], lhsT=m2s[:],
                                 rhs=hbn[:, 256:512],
                                 start=False, stop=True)
                e = ep.tile([128, 256], BF16, tag="e")
                nc.scalar.activation(e[:], z[:, 0:256], ACT.Exp,
                                     bias=vcol(V_BZ))
                # nm: one psum bank per window: [ct | At | xd | pad], one
                # accumulation chain per bank.
                nm = nm_psp.tile([128, 1024], F32, tag="nm")
                for i in range(2):
                    b = i * 512
                    isl = slice((w0 + i) * 128, (w0 + i + 1) * 128)
                    nc.tensor.matmul(nm[:, b:b + 128],
                                     lhsT=e[:, i * 128:(i + 1) * 128],
                                     rhs=wslice(K_U), start=True, stop=False)
                    nc.tensor.matmul(nm[:, b + 128:b + 256],
                                     lhsT=xT_sb[:, isl], rhs=wslice(K_WSL),
                                     start=False, stop=False)
                    nc.tensor.matmul(nm[:, b + 128:b + 256],
                                     lhsT=hbn[:, i * 128:(i + 1) * 128],
                                     rhs=w2n_s[:], start=False, stop=False)
                    nc.tensor.matmul(nm[:, b + 256:b + 384],
                                     lhsT=hbn[:, 256 + i * 128:
                                              256 + (i + 1) * 128],
                                     rhs=w2df_s[:], start=False, stop=False)
                    # bias rank-1 spanning At|xd, closes the chain
                    nc.tensor.matmul(nm[:, b + 128:b + 384],
                                     lhsT=ones_sb[:], rhs=rows_sb[:, 0:256],
                                     start=False, stop=True)
                nmv = nm[:, :].rearrange("p (i q) -> p i q", q=512)
                r = smallp.tile([128, 2], F32, tag="r")
                nc.vector.reciprocal(
                    r[:, :].rearrange("p (i u) -> p i u", u=1),
                    nmv[:, :, 127:128])
                t1 = t1p.tile([128, 256], BF16, tag="t1")
                for i in range(2):
                    if USE_ACT_T1:
                        nc.scalar.activation(
                            t1[:, i * 128:(i + 1) * 128],
                            nm[:, i * 512:i * 512 + 128],
                            ACT.Identity, scale=r[:, i:i + 1])
                    else:
                        nc.vector.tensor_scalar(
                            out=t1[:, i * 128:(i + 1) * 128],
                            in0=nm[:, i * 512:i * 512 + 128],
                            scalar1=r[:, i:i + 1], scalar2=None,
                            op0=OP.mult)
                u = up.tile([128, 256], BF16, tag="u")
                nc.vector.tensor_tensor(
                    out=u[:, :].rearrange("p (i q) -> p i q", i=2),
                    in0=t1[:, :].rearrange("p (i q) -> p i q", i=2),
                    in1=nmv[:, :, 256:384], op=OP.mult)
                o = outp.tile([128, 256], F32, tag="o")
                nc.vector.tensor_tensor(
                    out=o[:, :].rearrange("p (i q) -> p i q", i=2),
                    in0=u[:, :].rearrange("p (i q) -> p i q", i=2),
                    in1=nmv[:, :, 128:256], op=OP.add)
                nc.sync.dma_start(
                    out.ap()[w0 * 128:(w0 + 2) * 128, :]
                       .rearrange("(i p) f -> p i f", i=2),
                    o[:, :].rearrange("p (i f) -> p i f", i=2))

    nc.compile()
    return nc


def pack_edges(cfg: Cfg, src, dst, et):
    """Slot assignment. Returns (off [128, W*TT] int32 per core list,
    sel [128, W*NOV] f32 per core list). Raises if OV capacity exceeded."""
    C, W, KI, OV, TPT, TT, npc = (cfg.C, cfg.W, cfg.KI, cfg.OV, cfg.TPT,
                                  cfg.TT, cfg.npc)
    NOV = 2 * OV
    E = src.shape[0]
    core = dst // npc
    ldst = dst - core * npc
    wdw = ldst >> 7
    j = ldst & 127

    # stable sort by (core, window, type, j)
    gkey = ((core.astype(np.int64) * W + wdw) * 2 + et)
    fkey = gkey * 128 + j
    order = np.argsort(fkey, kind="stable")
    fs = fkey[order]
    gs = gkey[order]
    js = j[order]
    srcs = src[order]

    # rank within (c,w,t,j)
    fcounts = np.bincount(fs, minlength=cfg.C * W * 2 * 128)
    fstarts = np.concatenate([[0], np.cumsum(fcounts)[:-1]])
    rank = np.arange(E, dtype=np.int64) - fstarts[fs]

    id_mask = rank < KI
    ov_mask = ~id_mask
    # overflow rank within (c,w,t)
    cum = np.cumsum(ov_mask)
    gcounts = np.bincount(gs, minlength=cfg.C * W * 2)
    gstarts = np.concatenate([[0], np.cumsum(gcounts)[:-1]])
    cum_at_start = np.where(gstarts > 0, cum[gstarts - 1], 0)
    ovr = cum - 1 - cum_at_start[gs]

    max_ov = int((ovr[ov_mask].max() + 1) if ov_mask.any() else 0)
    if max_ov > OV * 128:
        raise RuntimeError(f"overflow capacity exceeded: {max_ov} > {OV*128}")

    cores_s = (gs // (2 * W)).astype(np.int64)
    w_s = (gs // 2) % W
    ty_s = gs % 2

    # tile index within window and partition
    tile_idx = np.where(id_mask, rank, KI + (ovr >> 7))
    part = np.where(id_mask, js, ovr & 127)
    col = w_s * TT + ty_s * TPT + tile_idx

    ZROW = cfg.N
    off = np.full((C, 128, W * TT), ZROW, np.int32)
    off[cores_s, part, col] = srcs

    sel = np.full((C, 128, W * NOV), -1.0, np.float32)
    ov_idx = np.nonzero(ov_mask)[0]
    scol = (w_s[ov_idx] * NOV + ty_s[ov_idx] * OV
            + (ovr[ov_idx] >> 7))
    sel[cores_s[ov_idx], ovr[ov_idx] & 127, scol] = js[ov_idx].astype(
        np.float32)
    return off, sel


def prep_inputs(cfg: Cfg, x, edge_index, edge_type, w_sl, b_sl,
                w1_n, b1_n, gamma_n, beta_n, w2_n, b2_n,
                w1_d, b1_d, gamma_d, beta_d, w2_d, b2_d,
                w_gat, b_gat):
    C, npc, npad = cfg.C, cfg.npc, cfg.npad
    x = np.asarray(x, np.float32)
    src = np.asarray(edge_index[0], np.int64).astype(np.int64)
    dst = np.asarray(edge_index[1], np.int64).astype(np.int64)
    et = np.asarray(edge_type, np.int64).astype(np.int64)

    off, sel = pack_edges(cfg, src, dst, et)

    xbf = np.vstack([x, np.zeros((1, 128), np.float32)]).astype(
        F8 if MSG_FP8 else BF)
    msgs = [np.ascontiguousarray(xbf[off[c]].reshape(128, -1))
            for c in range(C)]

    xTs = []
    xselfs = []
    W = cfg.W
    for c in range(C):
        xp = np.zeros((npad, 128), np.float32)
        xp[:npc] = x[c * npc:(c + 1) * npc]
        xTs.append(np.ascontiguousarray(xp.T).astype(BF))
        xselfs.append(np.ascontiguousarray(
            xp.reshape(W, 128, 128).transpose(1, 0, 2)
              .reshape(128, npad)).astype(BF))

    def bt(a):
        return np.ascontiguousarray(np.asarray(a, np.float64)).astype(BF)

    w_sl64 = np.asarray(w_sl, np.float64)
    w2n64 = np.asarray(w2_n, np.float64)
    w2d64 = np.asarray(w2_d, np.float64)
    wg = np.asarray(w_gat, np.float64)
    wg0, wg1, wg2 = wg[:, 0:128], wg[:, 128:256], wg[:, 256:384]

    wcols = [
        bt(w_sl64.T), bt(np.asarray(w1_n).T), bt(np.asarray(w1_d).T),
        bt(w2n64.T), bt(w2d64[::-1, :].T),
        bt((wg0 @ w_sl64).T), bt((wg1 @ w2n64).T), bt((wg2 @ w2d64).T),
        bt(np.triu(np.ones((128, 128), np.float32))),
        bt(np.eye(128, dtype=np.float32)),
    ]
    wpack = np.concatenate(wcols, axis=1)

    rows = np.concatenate([
        (np.asarray(b_sl, np.float64) + np.asarray(b2_n, np.float64))[None, :],
        np.asarray(b2_d, np.float64)[::-1][None, :],
    ], axis=1).astype(BF)

    bz = (np.asarray(b_gat, np.float64) + wg0 @ np.asarray(b_sl, np.float64)
          + wg1 @ np.asarray(b2_n, np.float64)
          + wg2 @ np.asarray(b2_d, np.float64))
    vecs = np.stack([
        np.asarray(b1_n, np.float64), np.asarray(b1_d, np.float64),
        np.asarray(gamma_n, np.float64), np.asarray(beta_n, np.float64),
        np.asarray(gamma_d, np.float64), np.asarray(beta_d, np.float64),
        bz,
    ], axis=1).astype(np.float32)

    in_maps = []
    for c in range(C):
        m = {
            "msgs": msgs[c],
            "xT": xTs[c],
            "xself": xselfs[c],
            "sel": np.ascontiguousarray(sel[c]),
            "wpack": wpack,
            "rows": rows,
            "vecs": vecs,
            "iota128": np.broadcast_to(
                np.arange(128, dtype=np.float32)[None, :],
                (128, 128)).copy(),
        }
        if MSG_FP8:
            m["ipair"] = np.concatenate(
                [np.eye(128, dtype=np.float32)] * 2, axis=1).astype(F8)
        in_maps.append(m)
    return in_maps


_BUILD_CACHE = {}


def run(cfg: Cfg, inputs: dict, **run_kwargs):
    in_maps = None
    while True:
        try:
            in_maps = prep_inputs(cfg, **inputs)
            break
        except RuntimeError:
            cfg = Cfg(cfg.N, cfg.E, cfg.C, cfg.KI, cfg.OV + 1)
    key = (cfg.N, cfg.E, cfg.C, cfg.KI, cfg.OV,
           USE_TTR, USE_STT, USE_POOL_HBN, USE_ACT_T1, MSG_FP8)
    if key not in _BUILD_CACHE:
        _BUILD_CACHE[key] = build(cfg)
    nc = _BUILD_CACHE[key]
    res = run_bass_kernel_spmd(nc, in_maps, core_ids=list(range(cfg.C)),
                               **run_kwargs)
    outs = [res.results[c]["out"][:cfg.npc] for c in range(cfg.C)]
    return np.concatenate(outs, axis=0).astype(np.float32), res


def kernel(**inputs):
    out, _ = run(CFG, inputs)
    return out


# revision 55
# speedup vs baseline: 2.1465x; 1.0429x over previous
"""Trainium2 Bass kernel for a 2-relation GIN-style GNN message-passing layer.

Full (unsharded) inputs in, full output out. Internally:
  - nodes sharded across 8 NeuronCores (12500/core, padded to 12544 = 98
    windows of 128); edges partitioned by destination-node shard (CPU prep).
  - per (window, relation), edges are packed into fixed tiles of 128 slots:
      * KI "identity" tiles: the t-th edge of destination j sits in
        partition j of tile t, so segment-sum over a tile is a plain
        transpose-accumulate: matmul(lhsT=msg_tile, rhs=I128). Empty slots
        gather a zero row of x.
      * OV "overflow" one-hot tiles for edges beyond KI per destination:
        matmul with a one-hot scatter matrix S built on-device via is_equal
        (padding slots sel=-1 give zero columns).
  - per-edge source rows are pre-gathered on CPU into a bf16 stream so the
    device sees only contiguous DMA.
  - BatchNorm batch stats are computed bias-free (bias folded analytically
    into the post-BN shift), via fused copy+row-sum (scalar engine
    accumulate) and fused square+reduce (DVE tensor_tensor_reduce), and
    AllReduce'd across the 8 cores in-kernel.
  - the BN scale is folded into the second-layer weights at runtime
    (requires gamma > 0, true for this model), so BN+ReLU is a single
    add+max op.
  - gate logits are computed with CPU-composed weights
    (w_gat_chunk @ w_branch), skipping the feature-major x_new_* tensors
    entirely; cumsum = matmul with triangular ones; flip folded into
    reversed weight rows; node-major outputs via data-stationary matmuls.
"""

import numpy as np
import ml_dtypes

import concourse.bass as bass
import concourse.mybir as mybir
import concourse.tile as tile
from concourse import bacc
from concourse.bass_utils import run_bass_kernel_spmd

F32 = mybir.dt.float32
BF16 = mybir.dt.bfloat16
FP8 = mybir.dt.float8e4
AX = mybir.AxisListType
OP = mybir.AluOpType
ACT = mybir.ActivationFunctionType
PM = mybir.MatmulPerfMode

BF = ml_dtypes.bfloat16
F8 = ml_dtypes.float8_e4m3


class Cfg:
    def __init__(self, N, E, C, KI, OV):
        self.N = N            # total nodes
        self.E = E            # total edges
        self.C = C            # cores
        self.F = 128
        self.KI = KI          # identity tiles per (window, type)
        self.OV = OV          # one-hot overflow tiles per (window, type)
        self.TPT = KI + OV    # tiles per type
        self.TT = 2 * self.TPT  # tiles per window (both types)
        assert N % C == 0
        self.npc = N // C                      # real nodes per core
        self.W = (self.npc + 127) // 128       # windows per core
        assert self.W % 2 == 0
        self.WB = self.W // 2                  # 2-window iterations
        self.npad = self.W * 128               # padded nodes per core


CFG = Cfg(N=100000, E=1600000, C=8, KI=10, OV=1)

# column layout of the "vecs" [128, 7] f32 input
(V_B1N, V_B1D, V_GN, V_BN, V_GD, V_BD, V_BZ) = range(7)

# column layout of wpack [128, 128*10] bf16
(K_WSL, K_W1N, K_W1D, K_W2N, K_W2DF, K_M0T, K_M1T, K_M2T, K_U, K_I) = range(10)

BN_EPS = 1e-5


USE_TTR = False       # tensor_tensor_reduce hangs TRN2 HW via this path
USE_STT = True        # fused square+accumulate via scalar_tensor_tensor
USE_POOL_HBN = False  # gpsimd tensor ops are ~8x slower than modeled
USE_ACT_T1 = True     # t1 via ACT Identity+scale (else DVE tensor_scalar)
MSG_FP8 = True        # fp8e4m3 message stream + DoubleRow paired matmuls


def build(cfg: Cfg):
    nc = bacc.Bacc("TRN2", target_bir_lowering=False, debug=False,
                   num_devices=cfg.C)
    W, WB, KI, OV, TPT, TT, npad = (cfg.W, cfg.WB, cfg.KI, cfg.OV,
                                    cfg.TPT, cfg.TT, cfg.npad)
    NOV = 2 * OV   # overflow tiles per window (both types)

    MDT = FP8 if MSG_FP8 else BF16
    msgs = nc.dram_tensor("msgs", [128, W * TT * 128], MDT,
                          kind="ExternalInput")
    xself = nc.dram_tensor("xself", [128, npad], BF16, kind="ExternalInput")
    if MSG_FP8:
        ipair = nc.dram_tensor("ipair", [128, 256], FP8,
                               kind="ExternalInput")
    xT = nc.dram_tensor("xT", [128, npad], BF16, kind="ExternalInput")
    sel = nc.dram_tensor("sel", [128, W * NOV], F32, kind="ExternalInput")
    wpack = nc.dram_tensor("wpack", [128, 128 * 10], BF16, kind="ExternalInput")
    rows = nc.dram_tensor("rows", [1, 256], BF16, kind="ExternalInput")
    vecs = nc.dram_tensor("vecs", [128, 7], F32, kind="ExternalInput")
    iota_in = nc.dram_tensor("iota128", [128, 128], F32, kind="ExternalInput")
    out = nc.dram_tensor("out", [npad, 128], F32, kind="ExternalOutput")

    with tile.TileContext(nc) as tc:
        with (
            tc.tile_pool(name="res", bufs=1) as res,
            tc.tile_pool(name="msgp", bufs=4) as msgp,
            tc.tile_pool(name="sp", bufs=4) as sp,
            tc.tile_pool(name="hxp", bufs=4) as hxp,
            tc.tile_pool(name="sqp", bufs=3) as sqp,
            tc.tile_pool(name="smallp", bufs=8) as smallp,
            tc.tile_pool(name="dram", bufs=1, space="DRAM") as dram,
            tc.tile_pool(name="hbnp", bufs=3) as hbnp,
            tc.tile_pool(name="ep", bufs=3) as ep,
            tc.tile_pool(name="t1p", bufs=3) as t1p,
            tc.tile_pool(name="up", bufs=3) as up,
            tc.tile_pool(name="outp", bufs=3) as outp,
        ):
            # ---------- resident loads ----------
            xT_sb = res.tile([128, npad], BF16)
            nc.sync.dma_start(xT_sb[:], xT.ap())
            xself_sb = res.tile([128, npad], BF16)
            nc.sync.dma_start(xself_sb[:], xself.ap())
            sel_sb = res.tile([128, W * NOV], F32)
            nc.sync.dma_start(sel_sb[:], sel.ap())
            wp = res.tile([128, 128 * 10], BF16)
            nc.sync.dma_start(wp[:], wpack.ap())
            rows_sb = res.tile([1, 256], BF16)
            nc.sync.dma_start(rows_sb[:], rows.ap())
            vec = res.tile([128, 7], F32)
            nc.sync.dma_start(vec[:], vecs.ap())
            iota_sb = res.tile([128, 128], F32)
            nc.sync.dma_start(iota_sb[:], iota_in.ap())
            if MSG_FP8:
                ipair_sb = res.tile([128, 256], FP8)
                nc.sync.dma_start(ipair_sb[:], ipair.ap())

            h1n_sb = res.tile([128, npad], BF16)
            h1d_sb = res.tile([128, npad], BF16)
            ones_sb = res.tile([1, 128], BF16)
            nc.vector.memset(ones_sb[:], 1.0)
            stat_s = res.tile([128, 2 * WB], F32)   # sums (ACT accum)
            stat_q = res.tile([128, 2 * WB], F32)   # sumsq (DVE accum)
            junk = res.tile([128, 8], F32)          # hx accum sink
            # runtime BN-folded params
            cvec = res.tile([128, 2], F32)          # relu shift per branch
            w2n_s = res.tile([128, 128], BF16)
            w2df_s = res.tile([128, 128], BF16)
            m1s = res.tile([128, 128], BF16)
            m2s = res.tile([128, 128], BF16)

            def wslice(k):
                return wp[:, k * 128:(k + 1) * 128]

            def vcol(k):
                return vec[:, k:k + 1]

            # ---------- phase A: aggregate + first linear + stats ----------
            with (
                tc.tile_pool(name="agg_ps", bufs=3, space="PSUM") as agg_psp,
                tc.tile_pool(name="h1_ps", bufs=3, space="PSUM") as h1_psp,
            ):
              for wb in range(WB):
                w0 = 2 * wb
                msg = msgp.tile([128, 2 * TT * 128], MDT, tag="msg")
                nc.sync.dma_start(
                    msg[:, :],
                    msgs.ap()[:, w0 * TT * 128:(w0 + 2) * TT * 128])
                # one-hot S for overflow tiles of both windows
                S = sp.tile([128, 2 * NOV * 128], FP8 if MSG_FP8 else BF16,
                            tag="S")
                nc.vector.tensor_tensor(
                    out=S[:, :].rearrange("p (t j) -> p t j", j=128),
                    in0=iota_sb[:, :].rearrange("p (x j) -> p x j", x=1)
                        .to_broadcast([128, 2 * NOV, 128]),
                    in1=sel_sb[:, w0 * NOV:(w0 + 2) * NOV]
                        .to_broadcast([128, 2 * NOV, 128]),
                    op=OP.is_equal,
                )
                # agg psum layout: [w0_n | w1_n | w0_d | w1_d] (128 each).
                # One accumulation chain for the whole bank: first matmul
                # start=True, last stop=True; each byte is lazily zeroed on
                # its first write after start.
                agg = agg_psp.tile([128, 512], F32, tag="agg")
                first = True
                for i in range(2):
                    mbase = i * TT * 128
                    for ty in range(2):
                        dst_sl = slice((2 * ty + i) * 128,
                                       (2 * ty + i + 1) * 128)
                        tbase = mbase + ty * TPT * 128
                        last_grp = (i == 1 and ty == 1)
                        # self-edge: fold +x into the aggregate via a
                        # transpose-matmul of the resident node-major x
                        nc.tensor.matmul(
                            agg[:, dst_sl],
                            lhsT=xself_sb[:, (w0 + i) * 128:
                                          (w0 + i + 1) * 128],
                            rhs=wslice(K_I),
                            start=first, stop=False)
                        first = False
                        if MSG_FP8:
                            # DoubleRow: two 128-slot tiles per matmul
                            for t in range(0, KI - 1, 2):
                                a = tbase + t * 128
                                nc.tensor.matmul(
                                    agg[:, dst_sl],
                                    lhsT=msg[:, a:a + 256].rearrange(
                                        "p (t j) -> p t j", t=2),
                                    rhs=ipair_sb[:, :].rearrange(
                                        "p (t j) -> p t j", t=2),
                                    perf_mode=PM.DoubleRow,
                                    start=first, stop=False)
                                first = False
                            if KI % 2:
                                a = tbase + (KI - 1) * 128
                                nc.tensor.matmul(
                                    agg[:, dst_sl], lhsT=msg[:, a:a + 128],
                                    rhs=ipair_sb[:, 0:128],
                                    start=first, stop=False)
                                first = False
                            sbase = (i * 2 + ty) * OV * 128
                            for t in range(0, OV - 1, 2):
                                a = tbase + (KI + t) * 128
                                s = sbase + t * 128
                                nc.tensor.matmul(
                                    agg[:, dst_sl],
                                    lhsT=msg[:, a:a + 256].rearrange(
                                        "p (t j) -> p t j", t=2),
                                    rhs=S[:, s:s + 256].rearrange(
                                        "p (t j) -> p t j", t=2),
                                    perf_mode=PM.DoubleRow,
                                    start=False,
                                    stop=(last_grp and t == OV - 2))
                            if OV % 2:
                                a = tbase + (KI + OV - 1) * 128
                                s = sbase + (OV - 1) * 128
                                nc.tensor.matmul(
                                    agg[:, dst_sl], lhsT=msg[:, a:a + 128],
                                    rhs=S[:, s:s + 128],
                                    start=False, stop=last_grp)
                        else:
                            for t in range(KI):
                                nc.tensor.matmul(
                                    agg[:, dst_sl],
                                    lhsT=msg[:, tbase + t * 128:
                                             tbase + (t + 1) * 128],
                                    rhs=wslice(K_I),
                                    start=first, stop=False)
                                first = False
                            for t in range(OV):
                                scol = ((i * 2 + ty) * OV + t) * 128
                                nc.tensor.matmul(
                                    agg[:, dst_sl],
                                    lhsT=msg[:, tbase + (KI + t) * 128:
                                             tbase + (KI + t + 1) * 128],
                                    rhs=S[:, scol:scol + 128],
                                    start=False,
                                    stop=(last_grp and t == OV - 1))
                # hx = agg (self-edges already added x); psum -> sbuf bf16.
                # Same instruction shape as the h1 copies below (Identity +
                # accum) so the ACT engine does not reload its table.
                hx = hxp.tile([128, 512], BF16, tag="hx")
                nc.scalar.activation(hx[:, :], agg[:, :], ACT.Identity,
                                     accum_out=junk[:, wb % 8:wb % 8 + 1])
                h1 = h1_psp.tile([128, 512], F32, tag="h1")
                nc.tensor.matmul(h1[:, 0:256], lhsT=wslice(K_W1N),
                                 rhs=hx[:, 0:256], start=True, stop=False)
                nc.tensor.matmul(h1[:, 256:512], lhsT=wslice(K_W1D),
                                 rhs=hx[:, 256:512], start=False, stop=True)
                # copy psum -> resident bf16 (no bias!) + row-sums on ACT
                nsl = slice(w0 * 128, (w0 + 2) * 128)
                nc.scalar.activation(
                    h1n_sb[:, nsl], h1[:, 0:256], ACT.Identity,
                    accum_out=stat_s[:, 2 * wb:2 * wb + 1])
                nc.scalar.activation(
                    h1d_sb[:, nsl], h1[:, 256:512], ACT.Identity,
                    accum_out=stat_s[:, 2 * wb + 1:2 * wb + 2])
                # sum of squares from the bf16 copies on DVE (2x mode)
                sq = sqp.tile([128, 512], BF16, tag="sq")
                if USE_STT:
                    nc.vector.scalar_tensor_tensor(
                        out=sq[:, 0:256], in0=h1n_sb[:, nsl], scalar=1.0,
                        in1=h1n_sb[:, nsl], op0=OP.mult, op1=OP.mult,
                        accum_out=stat_q[:, 2 * wb:2 * wb + 1])
                    nc.vector.scalar_tensor_tensor(
                        out=sq[:, 256:512], in0=h1d_sb[:, nsl], scalar=1.0,
                        in1=h1d_sb[:, nsl], op0=OP.mult, op1=OP.mult,
                        accum_out=stat_q[:, 2 * wb + 1:2 * wb + 2])
                elif USE_TTR:
                    nc.vector.tensor_tensor_reduce(
                        out=sq[:, 0:256], in0=h1n_sb[:, nsl],
                        in1=h1n_sb[:, nsl],
                        scale=1.0, scalar=0.0, op0=OP.mult, op1=OP.add,
                        accum_out=stat_q[:, 2 * wb:2 * wb + 1])
                    nc.vector.tensor_tensor_reduce(
                        out=sq[:, 256:512], in0=h1d_sb[:, nsl],
                        in1=h1d_sb[:, nsl],
                        scale=1.0, scalar=0.0, op0=OP.mult, op1=OP.add,
                        accum_out=stat_q[:, 2 * wb + 1:2 * wb + 2])
                else:
                    nc.vector.tensor_tensor(sq[:, 0:256], h1n_sb[:, nsl],
                                            h1n_sb[:, nsl], op=OP.mult)
                    nc.vector.tensor_tensor(sq[:, 256:512], h1d_sb[:, nsl],
                                            h1d_sb[:, nsl], op=OP.mult)
                    nc.vector.reduce_sum(
                        out=stat_q[:, 2 * wb:2 * wb + 1],
                        in_=sq[:, 0:256], axis=AX.X)
                    nc.vector.reduce_sum(
                        out=stat_q[:, 2 * wb + 1:2 * wb + 2],
                        in_=sq[:, 256:512], axis=AX.X)

            # ---------- stats reduce + allreduce + BN params ----------
            sums = smallp.tile([128, 4], F32, tag="sums")
            # col order: [sum_n, sumsq_n, sum_d, sumsq_d]
            for br in range(2):
                nc.vector.reduce_sum(
                    out=sums[:, 2 * br:2 * br + 1],
                    in_=stat_s[:, :].rearrange("p (w k) -> p w k", k=2)
                        [:, :, br],
                    axis=AX.X)
                nc.vector.reduce_sum(
                    out=sums[:, 2 * br + 1:2 * br + 2],
                    in_=stat_q[:, :].rearrange("p (w k) -> p w k", k=2)
                        [:, :, br],
                    axis=AX.X)
            cc_in = dram.tile([128, 4], F32)
            cc_out = dram.tile([128, 4], F32)
            nc.gpsimd.dma_start(cc_in[:], sums[:])
            nc.gpsimd.collective_compute(
                "AllReduce", OP.add,
                replica_groups=[list(range(cfg.C))],
                ins=[cc_in[:].opt()], outs=[cc_out[:].opt()],
            )
            gsums = smallp.tile([128, 4], F32, tag="gsums")
            nc.gpsimd.dma_start(gsums[:], cc_out[:])

            inv_n = 1.0 / cfg.N
            for br, (b1c, g_col, b_col) in enumerate([
                (V_B1N, V_GN, V_BN),
                (V_B1D, V_GD, V_BD),
            ]):
                mean = smallp.tile([128, 1], F32, tag="mean")
                nc.vector.tensor_scalar(
                    out=mean[:], in0=gsums[:, 2 * br:2 * br + 1],
                    scalar1=inv_n, scalar2=None, op0=OP.mult)
                msq = smallp.tile([128, 1], F32, tag="msq")
                nc.vector.tensor_tensor(msq[:], mean[:], mean[:], op=OP.mult)
                # var = gsumsq*inv_n - mean^2 (fused)
                var = smallp.tile([128, 1], F32, tag="var")
                nc.vector.tensor_scalar(
                    out=var[:], in0=gsums[:, 2 * br + 1:2 * br + 2],
                    scalar1=inv_n, scalar2=msq[:], op0=OP.mult,
                    op1=OP.subtract)
                # rstd = 1/sqrt(var + eps); scale = gamma * rstd  (> 0)
                nc.vector.tensor_scalar(out=var[:], in0=var[:],
                                        scalar1=BN_EPS, scalar2=None,
                                        op0=OP.add)
                std = smallp.tile([128, 1], F32, tag="std")
                nc.scalar.activation(std[:], var[:], ACT.Sqrt)
                rstd = smallp.tile([128, 1], F32, tag="rstd")
                nc.vector.reciprocal(rstd[:], std[:])
                sc = smallp.tile([128, 1], F32, tag="sc")
                nc.vector.tensor_tensor(sc[:], vcol(g_col), rstd[:],
                                        op=OP.mult)
                # c = beta / scale - mean  (the w1 bias cancels inside BN)
                rsc = smallp.tile([128, 1], F32, tag="rsc")
                nc.vector.reciprocal(rsc[:], sc[:])
                nc.vector.tensor_scalar(
                    out=cvec[:, br:br + 1], in0=rsc[:],
                    scalar1=vcol(b_col), scalar2=mean[:],
                    op0=OP.mult, op1=OP.subtract)
                # fold scale into second-layer weights
                wsl2 = wslice(K_W2N) if br == 0 else wslice(K_W2DF)
                wdst = w2n_s if br == 0 else w2df_s
                nc.vector.tensor_scalar(out=wdst[:], in0=wsl2,
                                        scalar1=sc[:], scalar2=None,
                                        op0=OP.mult)
                msl = wslice(K_M1T) if br == 0 else wslice(K_M2T)
                mdst = m1s if br == 0 else m2s
                nc.vector.tensor_scalar(out=mdst[:], in0=msl,
                                        scalar1=sc[:], scalar2=None,
                                        op0=OP.mult)

            # ---------- phase C: BN/relu, gate, outputs ----------
            with (
                tc.tile_pool(name="z_ps", bufs=2, space="PSUM") as z_psp,
                tc.tile_pool(name="nm_ps", bufs=3, space="PSUM") as nm_psp,
            ):
              for wb in range(WB):
                w0 = 2 * wb
                nsl = slice(w0 * 128, (w0 + 2) * 128)
                hbn = hbnp.tile([128, 512], BF16, tag="hbn")
                heng = nc.gpsimd if USE_POOL_HBN else nc.vector
                heng.tensor_scalar(
                    out=hbn[:, 0:256], in0=h1n_sb[:, nsl],
                    scalar1=cvec[:, 0:1], scalar2=0.0,
                    op0=OP.add, op1=OP.max)
                heng.tensor_scalar(
                    out=hbn[:, 256:512], in0=h1d_sb[:, nsl],
                    scalar1=cvec[:, 1:2], scalar2=0.0,
                    op0=OP.add, op1=OP.max)
                # gate logits via composed weights (bank-padded psum tile)
                z = z_psp.tile([128, 512], F32, tag="z")
                nc.tensor.matmul(z[:, 0:256], lhsT=wslice(K_M0T),
                                 rhs=xT_sb[:, nsl], start=True, stop=False)
                nc.tensor.matmul(z[:, 0:256], lhsT=m1s[:], rhs=hbn[:, 0:256],
                                 start=False, stop=False)
                nc.tensor.matmul(z[:, 0:256], lhsT=m2s[:],
                                 rhs=hbn[:, 256:512],
                                 start=False, stop=True)
                e = ep.tile([128, 256], BF16, tag="e")
                nc.scalar.activation(e[:], z[:, 0:256], ACT.Exp,
                                     bias=vcol(V_BZ))
                # nm: one psum bank per window: [ct | At | xd | pad], one
                # accumulation chain per bank.
                nm = nm_psp.tile([128, 1024], F32, tag="nm")
                for i in range(2):
                    b = i * 512
                    isl = slice((w0 + i) * 128, (w0 + i + 1) * 128)
                    nc.tensor.matmul(nm[:, b:b + 128],
                                     lhsT=e[:, i * 128:(i + 1) * 128],
                                     rhs=wslice(K_U), start=True, stop=False)
                    nc.tensor.matmul(nm[:, b + 128:b + 256],
                                     lhsT=xT_sb[:, isl], rhs=wslice(K_WSL),
                                     start=False, stop=False)
                    nc.tensor.matmul(nm[:, b + 128:b + 256],
                                     lhsT=hbn[:, i * 128:(i + 1) * 128],
                                     rhs=w2n_s[:], start=False, stop=False)
                    nc.tensor.matmul(nm[:, b + 256:b + 384],
                                     lhsT=hbn[:, 256 + i * 128:
                                              256 + (i + 1) * 128],
                                     rhs=w2df_s[:], start=False, stop=False)
                    # bias rank-1 spanning At|xd, closes the chain
                    nc.tensor.matmul(nm[:, b + 128:b + 384],
                                     lhsT=ones_sb[:], rhs=rows_sb[:, 0:256],
                                     start=False, stop=True)
                nmv = nm[:, :].rearrange("p (i q) -> p i q", q=512)
                r = smallp.tile([128, 2], F32, tag="r")
                nc.vector.reciprocal(
                    r[:, :].rearrange("p (i u) -> p i u", u=1),
                    nmv[:, :, 127:128])
                t1 = t1p.tile([128, 256], BF16, tag="t1")
                for i in range(2):
                    if USE_ACT_T1:
                        nc.scalar.activation(
                            t1[:, i * 128:(i + 1) * 128],
                            nm[:, i * 512:i * 512 + 128],
                            ACT.Identity, scale=r[:, i:i + 1])
                    else:
                        nc.vector.tensor_scalar(
                            out=t1[:, i * 128:(i + 1) * 128],
                            in0=nm[:, i * 512:i * 512 + 128],
                            scalar1=r[:, i:i + 1], scalar2=None,
                            op0=OP.mult)
                u = up.tile([128, 256], BF16, tag="u")
                nc.vector.tensor_tensor(
                    out=u[:, :].rearrange("p (i q) -> p i q", i=2),
                    in0=t1[:, :].rearrange("p (i q) -> p i q", i=2),
                    in1=nmv[:, :, 256:384], op=OP.mult)
                o = outp.tile([128, 256], F32, tag="o")
                nc.vector.tensor_tensor(
                    out=o[:, :].rearrange("p (i q) -> p i q", i=2),
                    in0=u[:, :].rearrange("p (i q) -> p i q", i=2),
                    in1=nmv[:, :, 128:256], op=OP.add)
                nc.sync.dma_start(
                    out.ap()[w0 * 128:(w0 + 2) * 128, :]
                       .rearrange("(i p) f -> p i f", i=2),
                    o[:, :].rearrange("p (i f) -> p i f", i=2))

    nc.compile()
    return nc


def pack_edges(cfg: Cfg, src, dst, et):
    """Slot assignment. Returns (off [128, W*TT] int32 per core list,
    sel [128, W*NOV] f32 per core list). Raises if OV capacity exceeded."""
    C, W, KI, OV, TPT, TT, npc = (cfg.C, cfg.W, cfg.KI, cfg.OV, cfg.TPT,
                                  cfg.TT, cfg.npc)
    NOV = 2 * OV
    E = src.shape[0]
    core = dst // npc
    ldst = dst - core * npc
    wdw = ldst >> 7
    j = ldst & 127

    # stable sort by (core, window, type, j)
    gkey = ((core.astype(np.int64) * W + wdw) * 2 + et)
    fkey = gkey * 128 + j
    order = np.argsort(fkey, kind="stable")
    fs = fkey[order]
    gs = gkey[order]
    js = j[order]
    srcs = src[order]

    # rank within (c,w,t,j)
    fcounts = np.bincount(fs, minlength=cfg.C * W * 2 * 128)
    fstarts = np.concatenate([[0], np.cumsum(fcounts)[:-1]])
    rank = np.arange(E, dtype=np.int64) - fstarts[fs]

    id_mask = rank < KI
    ov_mask = ~id_mask
    # overflow rank within (c,w,t)
    cum = np.cumsum(ov_mask)
    gcounts = np.bincount(gs, minlength=cfg.C * W * 2)
    gstarts = np.concatenate([[0], np.cumsum(gcounts)[:-1]])
    cum_at_start = np.where(gstarts > 0, cum[gstarts - 1], 0)
    ovr = cum - 1 - cum_at_start[gs]

    max_ov = int((ovr[ov_mask].max() + 1) if ov_mask.any() else 0)
    if max_ov > OV * 128:
        raise RuntimeError(f"overflow capacity exceeded: {max_ov} > {OV*128}")

    cores_s = (gs // (2 * W)).astype(np.int64)
    w_s = (gs // 2) % W
    ty_s = gs % 2

    # tile index within window and partition
    tile_idx = np.where(id_mask, rank, KI + (ovr >> 7))
    part = np.where(id_mask, js, ovr & 127)
    col = w_s * TT + ty_s * TPT + tile_idx

    ZROW = cfg.N
    off = np.full((C, 128, W * TT), ZROW, np.int32)
    off[cores_s, part, col] = srcs

    sel = np.full((C, 128, W * NOV), -1.0, np.float32)
    ov_idx = np.nonzero(ov_mask)[0]
    scol = (w_s[ov_idx] * NOV + ty_s[ov_idx] * OV
            + (ovr[ov_idx] >> 7))
    sel[cores_s[ov_idx], ovr[ov_idx] & 127, scol] = js[ov_idx].astype(
        np.float32)
    return off, sel


def prep_inputs(cfg: Cfg, x, edge_index, edge_type, w_sl, b_sl,
                w1_n, b1_n, gamma_n, beta_n, w2_n, b2_n,
                w1_d, b1_d, gamma_d, beta_d, w2_d, b2_d,
                w_gat, b_gat):
    C, npc, npad = cfg.C, cfg.npc, cfg.npad
    x = np.asarray(x, np.float32)
    src = np.asarray(edge_index[0], np.int64).astype(np.int64)
    dst = np.asarray(edge_index[1], np.int64).astype(np.int64)
    et = np.asarray(edge_type, np.int64).astype(np.int64)

    off, sel = pack_edges(cfg, src, dst, et)

    xbf = np.vstack([x, np.zeros((1, 128), np.float32)]).astype(
        F8 if MSG_FP8 else BF)
    msgs = [np.ascontiguousarray(xbf[off[c]].reshape(128, -1))
            for c in range(C)]

    xTs = []
    xselfs = []
    W = cfg.W
    for c in range(C):
        xp = np.zeros((npad, 128), np.float32)
        xp[:npc] = x[c * npc:(c + 1) * npc]
        xTs.append(np.ascontiguousarray(xp.T).astype(BF))
        xselfs.append(np.ascontiguousarray(
            xp.reshape(W, 128, 128).transpose(1, 0, 2)
              .reshape(128, npad)).astype(BF))

    def bt(a):
        return np.ascontiguousarray(np.asarray(a, np.float64)).astype(BF)

    w_sl64 = np.asarray(w_sl, np.float64)
    w2n64 = np.asarray(w2_n, np.float64)
    w2d64 = np.asarray(w2_d, np.float64)
    wg = np.asarray(w_gat, np.float64)
    wg0, wg1, wg2 = wg[:, 0:128], wg[:, 128:256], wg[:, 256:384]

    wcols = [
        bt(w_sl64.T), bt(np.asarray(w1_n).T), bt(np.asarray(w1_d).T),
        bt(w2n64.T), bt(w2d64[::-1, :].T),
        bt((wg0 @ w_sl64).T), bt((wg1 @ w2n64).T), bt((wg2 @ w2d64).T),
        bt(np.triu(np.ones((128, 128), np.float32))),
        bt(np.eye(128, dtype=np.float32)),
    ]
    wpack = np.concatenate(wcols, axis=1)

    rows = np.concatenate([
        (np.asarray(b_sl, np.float64) + np.asarray(b2_n, np.float64))[None, :],
        np.asarray(b2_d, np.float64)[::-1][None, :],
    ], axis=1).astype(BF)

    bz = (np.asarray(b_gat, np.float64) + wg0 @ np.asarray(b_sl, np.float64)
          + wg1 @ np.asarray(b2_n, np.float64)
          + wg2 @ np.asarray(b2_d, np.float64))
    vecs = np.stack([
        np.asarray(b1_n, np.float64), np.asarray(b1_d, np.float64),
        np.asarray(gamma_n, np.float64), np.asarray(beta_n, np.float64),
        np.asarray(gamma_d, np.float64), np.asarray(beta_d, np.float64),
        bz,
    ], axis=1).astype(np.float32)

    in_maps = []
    for c in range(C):
        m = {
            "msgs": msgs[c],
            "xT": xTs[c],
            "xself": xselfs[c],
            "sel": np.ascontiguousarray(sel[c]),
            "wpack": wpack,
            "rows": rows,
            "vecs": vecs,
            "iota128": np.broadcast_to(
                np.arange(128, dtype=np.float32)[None, :],
                (128, 128)).copy(),
        }
        if MSG_FP8:
            m["ipair"] = np.concatenate(
                [np.eye(128, dtype=np.float32)] * 2, axis=1).astype(F8)
        in_maps.append(m)
    return in_maps


_BUILD_CACHE = {}


def run(cfg: Cfg, inputs: dict, **run_kwargs):
    in_maps = None
    while True:
        try:
            in_maps = prep_inputs(cfg, **inputs)
            break
        except RuntimeError:
            cfg = Cfg(cfg.N, cfg.E, cfg.C, cfg.KI, cfg.OV + 1)
    key = (cfg.N, cfg.E, cfg.C, cfg.KI, cfg.OV,
           USE_TTR, USE_STT, USE_POOL_HBN, USE_ACT_T1, MSG_FP8)
    if key not in _BUILD_CACHE:
        _BUILD_CACHE[key] = build(cfg)
    nc = _BUILD_CACHE[key]
    res = run_bass_kernel_spmd(nc, in_maps, core_ids=list(range(cfg.C)),
                               **run_kwargs)
    outs = [res.results[c]["out"][:cfg.npc] for c in range(cfg.C)]
    return np.concatenate(outs, axis=0).astype(np.float32), res


def kernel(**inputs):
    out, _ = run(CFG, inputs)
    return out


# revision 58
# speedup vs baseline: 2.2788x; 1.0616x over previous
"""Trainium2 Bass kernel for a 2-relation GIN-style GNN message-passing layer.

Full (unsharded) inputs in, full output out. Internally:
  - nodes sharded across 8 NeuronCores (12500/core, padded to 12544 = 98
    windows of 128); edges partitioned by destination-node shard (CPU prep).
  - per (window, relation), edges are packed into fixed tiles of 128 slots:
      * KI "identity" tiles: the t-th edge of destination j sits in
        partition j of tile t, so segment-sum over a tile is a plain
        transpose-accumulate: matmul(lhsT=msg_tile, rhs=I128). Empty slots
        gather a zero row of x.
      * OV "overflow" one-hot tiles for edges beyond KI per destination:
        matmul with a one-hot scatter matrix S built on-device via is_equal
        (padding slots sel=-1 give zero columns).
  - per-edge source rows are pre-gathered on CPU into a bf16 stream so the
    device sees only contiguous DMA.
  - BatchNorm batch stats are computed bias-free (bias folded analytically
    into the post-BN shift), via fused copy+row-sum (scalar engine
    accumulate) and fused square+reduce (DVE tensor_tensor_reduce), and
    AllReduce'd across the 8 cores in-kernel.
  - the BN scale is folded into the second-layer weights at runtime
    (requires gamma > 0, true for this model), so BN+ReLU is a single
    add+max op.
  - gate logits are computed with CPU-composed weights
    (w_gat_chunk @ w_branch), skipping the feature-major x_new_* tensors
    entirely; cumsum = matmul with triangular ones; flip folded into
    reversed weight rows; node-major outputs via data-stationary matmuls.
"""

import numpy as np
import ml_dtypes

import concourse.bass as bass
import concourse.mybir as mybir
import concourse.tile as tile
from concourse import bacc
from concourse.bass_utils import run_bass_kernel_spmd

F32 = mybir.dt.float32
BF16 = mybir.dt.bfloat16
FP8 = mybir.dt.float8e4
AX = mybir.AxisListType
OP = mybir.AluOpType
ACT = mybir.ActivationFunctionType
PM = mybir.MatmulPerfMode

BF = ml_dtypes.bfloat16
F8 = ml_dtypes.float8_e4m3


class Cfg:
    def __init__(self, N, E, C, KI, OV):
        self.N = N            # total nodes
        self.E = E            # total edges
        self.C = C            # cores
        self.F = 128
        self.KI = KI          # identity tiles per (window, type)
        self.OV = OV          # one-hot overflow tiles per (window, type)
        self.TPT = KI + OV    # tiles per type
        self.TT = 2 * self.TPT  # tiles per window (both types)
        assert N % C == 0
        self.npc = N // C                      # real nodes per core
        self.W = (self.npc + 127) // 128       # windows per core
        assert self.W % 2 == 0
        self.WB = self.W // 2                  # 2-window iterations
        self.npad = self.W * 128               # padded nodes per core


CFG = Cfg(N=100000, E=1600000, C=8, KI=10, OV=1)

# column layout of the "vecs" [128, 7] f32 input
(V_B1N, V_B1D, V_GN, V_BN, V_GD, V_BD, V_BZ) = range(7)

# column layout of wpack [128, 128*10] bf16
(K_WSL, K_W1N, K_W1D, K_W2N, K_W2DF, K_M0T, K_M1T, K_M2T, K_U, K_I) = range(10)

BN_EPS = 1e-5


USE_TTR = False       # tensor_tensor_reduce hangs TRN2 HW via this path
USE_STT = True        # fused square+accumulate via scalar_tensor_tensor
USE_POOL_HBN = False  # gpsimd tensor ops are ~8x slower than modeled
USE_ACT_T1 = True     # t1 via ACT Identity+scale (else DVE tensor_scalar)
MSG_FP8 = True        # fp8e4m3 message stream + DoubleRow paired matmuls


def build(cfg: Cfg):
    nc = bacc.Bacc("TRN2", target_bir_lowering=False, debug=False,
                   num_devices=cfg.C)
    W, WB, KI, OV, TPT, TT, npad = (cfg.W, cfg.WB, cfg.KI, cfg.OV,
                                    cfg.TPT, cfg.TT, cfg.npad)
    NOV = 2 * OV   # overflow tiles per window (both types)

    MDT = FP8 if MSG_FP8 else BF16
    msgs = nc.dram_tensor("msgs", [128, W * TT * 128], MDT,
                          kind="ExternalInput")
    xself = nc.dram_tensor("xself", [128, npad], BF16, kind="ExternalInput")
    if MSG_FP8:
        ipair = nc.dram_tensor("ipair", [128, 256], FP8,
                               kind="ExternalInput")
    xT = nc.dram_tensor("xT", [128, npad], BF16, kind="ExternalInput")
    sel = nc.dram_tensor("sel", [128, W * NOV], F32, kind="ExternalInput")
    wpack = nc.dram_tensor("wpack", [128, 128 * 10], BF16, kind="ExternalInput")
    rows = nc.dram_tensor("rows", [1, 256], BF16, kind="ExternalInput")
    vecs = nc.dram_tensor("vecs", [128, 7], F32, kind="ExternalInput")
    iota_in = nc.dram_tensor("iota128", [128, 128], F32, kind="ExternalInput")
    out = nc.dram_tensor("out", [npad, 128], F32, kind="ExternalOutput")

    with tile.TileContext(nc) as tc:
        with (
            tc.tile_pool(name="res", bufs=1) as res,
            tc.tile_pool(name="msgp", bufs=4) as msgp,
            tc.tile_pool(name="sp", bufs=4) as sp,
            tc.tile_pool(name="hxp", bufs=4) as hxp,
            tc.tile_pool(name="sqp", bufs=3) as sqp,
            tc.tile_pool(name="smallp", bufs=8) as smallp,
            tc.tile_pool(name="dram", bufs=1, space="DRAM") as dram,
            tc.tile_pool(name="hbnp", bufs=3) as hbnp,
            tc.tile_pool(name="ep", bufs=3) as ep,
            tc.tile_pool(name="t1p", bufs=3) as t1p,
            tc.tile_pool(name="up", bufs=3) as up,
            tc.tile_pool(name="outp", bufs=3) as outp,
        ):
            # ---------- resident loads ----------
            xT_sb = res.tile([128, npad], BF16)
            nc.sync.dma_start(xT_sb[:], xT.ap())
            xself_sb = res.tile([128, npad], BF16)
            nc.sync.dma_start(xself_sb[:], xself.ap())
            sel_sb = res.tile([128, W * NOV], F32)
            nc.sync.dma_start(sel_sb[:], sel.ap())
            wp = res.tile([128, 128 * 10], BF16)
            nc.sync.dma_start(wp[:], wpack.ap())
            rows_sb = res.tile([1, 256], BF16)
            nc.sync.dma_start(rows_sb[:], rows.ap())
            vec = res.tile([128, 7], F32)
            nc.sync.dma_start(vec[:], vecs.ap())
            iota_sb = res.tile([128, 128], F32)
            nc.sync.dma_start(iota_sb[:], iota_in.ap())
            if MSG_FP8:
                ipair_sb = res.tile([128, 256], FP8)
                nc.sync.dma_start(ipair_sb[:], ipair.ap())

            h1n_sb = res.tile([128, npad], BF16)
            h1d_sb = res.tile([128, npad], BF16)
            ones_sb = res.tile([1, 128], BF16)
            nc.vector.memset(ones_sb[:], 1.0)
            stat_s = res.tile([128, 2 * WB], F32)   # sums (ACT accum)
            stat_q = res.tile([128, 2 * WB], F32)   # sumsq (DVE accum)
            junk = res.tile([128, 8], F32)          # hx accum sink
            # runtime BN-folded params
            cvec = res.tile([128, 2], F32)          # relu shift per branch
            w2n_s = res.tile([128, 128], BF16)
            w2df_s = res.tile([128, 128], BF16)
            m1s = res.tile([128, 128], BF16)
            m2s = res.tile([128, 128], BF16)

            def wslice(k):
                return wp[:, k * 128:(k + 1) * 128]

            def vcol(k):
                return vec[:, k:k + 1]

            # ---------- phase A: aggregate + first linear + stats ----------
            SPLITB = (4 * WB) // 5   # stats allreduce #1 covers wb < SPLITB
            cc1_in = dram.tile([128, 4], F32)
            cc1_out = dram.tile([128, 4], F32)
            cc2_in = dram.tile([128, 4], F32)
            cc2_out = dram.tile([128, 4], F32)
            sums1 = res.tile([128, 4], F32)
            sums2 = res.tile([128, 4], F32)

            def emit_stats_cc(sums, cc_in, cc_out, lo, hi):
                # col order: [sum_n, sumsq_n, sum_d, sumsq_d]
                for br in range(2):
                    nc.vector.reduce_sum(
                        out=sums[:, 2 * br:2 * br + 1],
                        in_=stat_s[:, 2 * lo:2 * hi].rearrange(
                            "p (w k) -> p w k", k=2)[:, :, br],
                        axis=AX.X)
                    nc.vector.reduce_sum(
                        out=sums[:, 2 * br + 1:2 * br + 2],
                        in_=stat_q[:, 2 * lo:2 * hi].rearrange(
                            "p (w k) -> p w k", k=2)[:, :, br],
                        axis=AX.X)
                nc.gpsimd.dma_start(cc_in[:], sums[:])
                nc.gpsimd.collective_compute(
                    "AllReduce", OP.add,
                    replica_groups=[list(range(cfg.C))],
                    ins=[cc_in[:].opt()], outs=[cc_out[:].opt()],
                )

            with (
                tc.tile_pool(name="agg_ps", bufs=4, space="PSUM") as agg_psp,
                tc.tile_pool(name="h1_ps", bufs=3, space="PSUM") as h1_psp,
            ):
              for wb in range(WB):
                w0 = 2 * wb
                msg = msgp.tile([128, 2 * TT * 128], MDT, tag="msg")
                nc.sync.dma_start(
                    msg[:, :],
                    msgs.ap()[:, w0 * TT * 128:(w0 + 2) * TT * 128])
                # one-hot S for overflow tiles of both windows
                S = sp.tile([128, 2 * NOV * 128], FP8 if MSG_FP8 else BF16,
                            tag="S")
                nc.vector.tensor_tensor(
                    out=S[:, :].rearrange("p (t j) -> p t j", j=128),
                    in0=iota_sb[:, :].rearrange("p (x j) -> p x j", x=1)
                        .to_broadcast([128, 2 * NOV, 128]),
                    in1=sel_sb[:, w0 * NOV:(w0 + 2) * NOV]
                        .to_broadcast([128, 2 * NOV, 128]),
                    op=OP.is_equal,
                )
                # agg psum layout: [w0_n | w1_n | w0_d | w1_d] (128 each).
                # One accumulation chain for the whole bank: first matmul
                # start=True, last stop=True; each byte is lazily zeroed on
                # its first write after start.
                agg = agg_psp.tile([128, 512], F32, tag="agg")
                first = True
                for i in range(2):
                    mbase = i * TT * 128
                    for ty in range(2):
                        dst_sl = slice((2 * ty + i) * 128,
                                       (2 * ty + i + 1) * 128)
                        tbase = mbase + ty * TPT * 128
                        last_grp = (i == 1 and ty == 1)
                        # self-edge: fold +x into the aggregate via a
                        # transpose-matmul of the resident node-major x
                        nc.tensor.matmul(
                            agg[:, dst_sl],
                            lhsT=xself_sb[:, (w0 + i) * 128:
                                          (w0 + i + 1) * 128],
                            rhs=wslice(K_I),
                            start=first, stop=False)
                        first = False
                        if MSG_FP8:
                            # DoubleRow: two 128-slot tiles per matmul
                            for t in range(0, KI - 1, 2):
                                a = tbase + t * 128
                                nc.tensor.matmul(
                                    agg[:, dst_sl],
                                    lhsT=msg[:, a:a + 256].rearrange(
                                        "p (t j) -> p t j", t=2),
                                    rhs=ipair_sb[:, :].rearrange(
                                        "p (t j) -> p t j", t=2),
                                    perf_mode=PM.DoubleRow,
                                    start=first, stop=False)
                                first = False
                            if KI % 2:
                                a = tbase + (KI - 1) * 128
                                nc.tensor.matmul(
                                    agg[:, dst_sl], lhsT=msg[:, a:a + 128],
                                    rhs=ipair_sb[:, 0:128],
                                    start=first, stop=False)
                                first = False
                            sbase = (i * 2 + ty) * OV * 128
                            for t in range(0, OV - 1, 2):
                                a = tbase + (KI + t) * 128
                                s = sbase + t * 128
                                nc.tensor.matmul(
                                    agg[:, dst_sl],
                                    lhsT=msg[:, a:a + 256].rearrange(
                                        "p (t j) -> p t j", t=2),
                                    rhs=S[:, s:s + 256].rearrange(
                                        "p (t j) -> p t j", t=2),
                                    perf_mode=PM.DoubleRow,
                                    start=False,
                                    stop=(last_grp and t == OV - 2))
                            if OV % 2:
                                a = tbase + (KI + OV - 1) * 128
                                s = sbase + (OV - 1) * 128
                                nc.tensor.matmul(
                                    agg[:, dst_sl], lhsT=msg[:, a:a + 128],
                                    rhs=S[:, s:s + 128],
                                    start=False, stop=last_grp)
                        else:
                            for t in range(KI):
                                nc.tensor.matmul(
                                    agg[:, dst_sl],
                                    lhsT=msg[:, tbase + t * 128:
                                             tbase + (t + 1) * 128],
                                    rhs=wslice(K_I),
                                    start=first, stop=False)
                                first = False
                            for t in range(OV):
                                scol = ((i * 2 + ty) * OV + t) * 128
                                nc.tensor.matmul(
                                    agg[:, dst_sl],
                                    lhsT=msg[:, tbase + (KI + t) * 128:
                                             tbase + (KI + t + 1) * 128],
                                    rhs=S[:, scol:scol + 128],
                                    start=False,
                                    stop=(last_grp and t == OV - 1))
                # hx = agg (self-edges already added x); psum -> sbuf bf16.
                # Same instruction shape as the h1 copies below (Identity +
                # accum) so the ACT engine does not reload its table.
                hx = hxp.tile([128, 512], BF16, tag="hx")
                nc.scalar.activation(hx[:, :], agg[:, :], ACT.Identity,
                                     accum_out=junk[:, wb % 8:wb % 8 + 1])
                h1 = h1_psp.tile([128, 512], F32, tag="h1")
                nc.tensor.matmul(h1[:, 0:256], lhsT=wslice(K_W1N),
                                 rhs=hx[:, 0:256], start=True, stop=False)
                nc.tensor.matmul(h1[:, 256:512], lhsT=wslice(K_W1D),
                                 rhs=hx[:, 256:512], start=False, stop=True)
                # copy psum -> resident bf16 (no bias!) + row-sums on ACT
                nsl = slice(w0 * 128, (w0 + 2) * 128)
                nc.scalar.activation(
                    h1n_sb[:, nsl], h1[:, 0:256], ACT.Identity,
                    accum_out=stat_s[:, 2 * wb:2 * wb + 1])
                nc.scalar.activation(
                    h1d_sb[:, nsl], h1[:, 256:512], ACT.Identity,
                    accum_out=stat_s[:, 2 * wb + 1:2 * wb + 2])
                # sum of squares from the bf16 copies on DVE (2x mode)
                sq = sqp.tile([128, 512], BF16, tag="sq")
                if USE_STT:
                    nc.vector.scalar_tensor_tensor(
                        out=sq[:, 0:256], in0=h1n_sb[:, nsl], scalar=1.0,
                        in1=h1n_sb[:, nsl], op0=OP.mult, op1=OP.mult,
                        accum_out=stat_q[:, 2 * wb:2 * wb + 1])
                    nc.vector.scalar_tensor_tensor(
                        out=sq[:, 256:512], in0=h1d_sb[:, nsl], scalar=1.0,
                        in1=h1d_sb[:, nsl], op0=OP.mult, op1=OP.mult,
                        accum_out=stat_q[:, 2 * wb + 1:2 * wb + 2])
                    if wb == SPLITB - 1:
                        emit_stats_cc(sums1, cc1_in, cc1_out, 0, SPLITB)
                elif USE_TTR:
                    nc.vector.tensor_tensor_reduce(
                        out=sq[:, 0:256], in0=h1n_sb[:, nsl],
                        in1=h1n_sb[:, nsl],
                        scale=1.0, scalar=0.0, op0=OP.mult, op1=OP.add,
                        accum_out=stat_q[:, 2 * wb:2 * wb + 1])
                    nc.vector.tensor_tensor_reduce(
                        out=sq[:, 256:512], in0=h1d_sb[:, nsl],
                        in1=h1d_sb[:, nsl],
                        scale=1.0, scalar=0.0, op0=OP.mult, op1=OP.add,
                        accum_out=stat_q[:, 2 * wb + 1:2 * wb + 2])
                else:
                    nc.vector.tensor_tensor(sq[:, 0:256], h1n_sb[:, nsl],
                                            h1n_sb[:, nsl], op=OP.mult)
                    nc.vector.tensor_tensor(sq[:, 256:512], h1d_sb[:, nsl],
                                            h1d_sb[:, nsl], op=OP.mult)
                    nc.vector.reduce_sum(
                        out=stat_q[:, 2 * wb:2 * wb + 1],
                        in_=sq[:, 0:256], axis=AX.X)
                    nc.vector.reduce_sum(
                        out=stat_q[:, 2 * wb + 1:2 * wb + 2],
                        in_=sq[:, 256:512], axis=AX.X)

            # ---------- stats reduce + allreduce + BN params ----------
            emit_stats_cc(sums2, cc2_in, cc2_out, SPLITB, WB)
            g1 = smallp.tile([128, 4], F32, tag="g1")
            nc.gpsimd.dma_start(g1[:], cc1_out[:])
            g2 = smallp.tile([128, 4], F32, tag="g2")
            nc.gpsimd.dma_start(g2[:], cc2_out[:])
            gsums = smallp.tile([128, 4], F32, tag="gsums")
            nc.vector.tensor_tensor(gsums[:], g1[:], g2[:], op=OP.add)

            inv_n = 1.0 / cfg.N
            for br, (b1c, g_col, b_col) in enumerate([
                (V_B1N, V_GN, V_BN),
                (V_B1D, V_GD, V_BD),
            ]):
                mean = smallp.tile([128, 1], F32, tag="mean")
                nc.vector.tensor_scalar(
                    out=mean[:], in0=gsums[:, 2 * br:2 * br + 1],
                    scalar1=inv_n, scalar2=None, op0=OP.mult)
                msq = smallp.tile([128, 1], F32, tag="msq")
                nc.vector.tensor_tensor(msq[:], mean[:], mean[:], op=OP.mult)
                # var = gsumsq*inv_n - mean^2 + eps (fused)
                var = smallp.tile([128, 1], F32, tag="var")
                nc.vector.tensor_scalar(
                    out=var[:], in0=gsums[:, 2 * br + 1:2 * br + 2],
                    scalar1=inv_n, scalar2=msq[:], op0=OP.mult,
                    op1=OP.subtract)
                nc.vector.tensor_scalar(out=var[:], in0=var[:],
                                        scalar1=BN_EPS, scalar2=None,
                                        op0=OP.add)
                std = smallp.tile([128, 1], F32, tag="std")
                nc.scalar.activation(std[:], var[:], ACT.Sqrt)
                rstd = smallp.tile([128, 1], F32, tag="rstd")
                nc.vector.reciprocal(rstd[:], std[:])
                sc = smallp.tile([128, 1], F32, tag="sc")
                nc.vector.tensor_tensor(sc[:], vcol(g_col), rstd[:],
                                        op=OP.mult)
                # c = beta / scale - mean  (the w1 bias cancels inside BN)
                rsc = smallp.tile([128, 1], F32, tag="rsc")
                nc.vector.reciprocal(rsc[:], sc[:])
                nc.vector.tensor_scalar(
                    out=cvec[:, br:br + 1], in0=rsc[:],
                    scalar1=vcol(b_col), scalar2=mean[:],
                    op0=OP.mult, op1=OP.subtract)
                # fold scale into second-layer weights (ACT + DVE in parallel)
                wsl2 = wslice(K_W2N) if br == 0 else wslice(K_W2DF)
                wdst = w2n_s if br == 0 else w2df_s
                nc.scalar.activation(wdst[:], wsl2, ACT.Identity,
                                     scale=sc[:])
                msl = wslice(K_M1T) if br == 0 else wslice(K_M2T)
                mdst = m1s if br == 0 else m2s
                nc.vector.tensor_scalar(out=mdst[:], in0=msl,
                                        scalar1=sc[:], scalar2=None,
                                        op0=OP.mult)

            # ---------- phase C: BN/relu, gate, outputs ----------
            with (
                tc.tile_pool(name="z_ps", bufs=2, space="PSUM") as z_psp,
                tc.tile_pool(name="nm_ps", bufs=3, space="PSUM") as nm_psp,
            ):
              for wb in range(WB):
                w0 = 2 * wb
                nsl = slice(w0 * 128, (w0 + 2) * 128)
                hbn = hbnp.tile([128, 512], BF16, tag="hbn")
                heng = nc.gpsimd if USE_POOL_HBN else nc.vector
                heng.tensor_scalar(
                    out=hbn[:, 0:256], in0=h1n_sb[:, nsl],
                    scalar1=cvec[:, 0:1], scalar2=0.0,
                    op0=OP.add, op1=OP.max)
                heng.tensor_scalar(
                    out=hbn[:, 256:512], in0=h1d_sb[:, nsl],
                    scalar1=cvec[:, 1:2], scalar2=0.0,
                    op0=OP.add, op1=OP.max)
                # gate logits via composed weights (bank-padded psum tile)
                z = z_psp.tile([128, 512], F32, tag="z")
                nc.tensor.matmul(z[:, 0:256], lhsT=wslice(K_M0T),
                                 rhs=xT_sb[:, nsl], start=True, stop=False)
                nc.tensor.matmul(z[:, 0:256], lhsT=m1s[:], rhs=hbn[:, 0:256],
                                 start=False, stop=False)
                nc.tensor.matmul(z[:, 0:256], lhsT=m2s[:],
                                 rhs=hbn[:, 256:512],
                                 start=False, stop=True)
                e = ep.tile([128, 256], BF16, tag="e")
                nc.scalar.activation(e[:], z[:, 0:256], ACT.Exp,
                                     bias=vcol(V_BZ))
                # nm: one psum bank per window: [ct | At | xd | pad], one
                # accumulation chain per bank.
                nm = nm_psp.tile([128, 1024], F32, tag="nm")
                for i in range(2):
                    b = i * 512
                    isl = slice((w0 + i) * 128, (w0 + i + 1) * 128)
                    nc.tensor.matmul(nm[:, b:b + 128],
                                     lhsT=e[:, i * 128:(i + 1) * 128],
                                     rhs=wslice(K_U), start=True, stop=False)
                    nc.tensor.matmul(nm[:, b + 128:b + 256],
                                     lhsT=xT_sb[:, isl], rhs=wslice(K_WSL),
                                     start=False, stop=False)
                    nc.tensor.matmul(nm[:, b + 128:b + 256],
                                     lhsT=hbn[:, i * 128:(i + 1) * 128],
                                     rhs=w2n_s[:], start=False, stop=False)
                    nc.tensor.matmul(nm[:, b + 256:b + 384],
                                     lhsT=hbn[:, 256 + i * 128:
                                              256 + (i + 1) * 128],
                                     rhs=w2df_s[:], start=False, stop=False)
                    # bias rank-1 spanning At|xd, closes the chain
                    nc.tensor.matmul(nm[:, b + 128:b + 384],
                                     lhsT=ones_sb[:], rhs=rows_sb[:, 0:256],
                                     start=False, stop=True)
                nmv = nm[:, :].rearrange("p (i q) -> p i q", q=512)
                r = smallp.tile([128, 2], F32, tag="r")
                nc.vector.reciprocal(
                    r[:, :].rearrange("p (i u) -> p i u", u=1),
                    nmv[:, :, 127:128])
                t1 = t1p.tile([128, 256], BF16, tag="t1")
                for i in range(2):
                    if USE_ACT_T1:
                        nc.scalar.activation(
                            t1[:, i * 128:(i + 1) * 128],
                            nm[:, i * 512:i * 512 + 128],
                            ACT.Identity, scale=r[:, i:i + 1])
                    else:
                        nc.vector.tensor_scalar(
                            out=t1[:, i * 128:(i + 1) * 128],
                            in0=nm[:, i * 512:i * 512 + 128],
                            scalar1=r[:, i:i + 1], scalar2=None,
                            op0=OP.mult)
                u = up.tile([128, 256], BF16, tag="u")
                nc.vector.tensor_tensor(
                    out=u[:, :].rearrange("p (i q) -> p i q", i=2),
                    in0=t1[:, :].rearrange("p (i q) -> p i q", i=2),
                    in1=nmv[:, :, 256:384], op=OP.mult)
                o = outp.tile([128, 256], F32, tag="o")
                nc.vector.tensor_tensor(
                    out=o[:, :].rearrange("p (i q) -> p i q", i=2),
                    in0=u[:, :].rearrange("p (i q) -> p i q", i=2),
                    in1=nmv[:, :, 128:256], op=OP.add)
                nc.sync.dma_start(
                    out.ap()[w0 * 128:(w0 + 2) * 128, :]
                       .rearrange("(i p) f -> p i f", i=2),
                    o[:, :].rearrange("p (i f) -> p i f", i=2))

    nc.compile()
    return nc


def pack_edges(cfg: Cfg, src, dst, et):
    """Slot assignment. Returns (off [128, W*TT] int32 per core list,
    sel [128, W*NOV] f32 per core list). Raises if OV capacity exceeded."""
    C, W, KI, OV, TPT, TT, npc = (cfg.C, cfg.W, cfg.KI, cfg.OV, cfg.TPT,
                                  cfg.TT, cfg.npc)
    NOV = 2 * OV
    E = src.shape[0]
    core = dst // npc
    ldst = dst - core * npc
    wdw = ldst >> 7
    j = ldst & 127

    # stable sort by (core, window, type, j)
    gkey = ((core.astype(np.int64) * W + wdw) * 2 + et)
    fkey = gkey * 128 + j
    order = np.argsort(fkey, kind="stable")
    fs = fkey[order]
    gs = gkey[order]
    js = j[order]
    srcs = src[order]

    # rank within (c,w,t,j)
    fcounts = np.bincount(fs, minlength=cfg.C * W * 2 * 128)
    fstarts = np.concatenate([[0], np.cumsum(fcounts)[:-1]])
    rank = np.arange(E, dtype=np.int64) - fstarts[fs]

    id_mask = rank < KI
    ov_mask = ~id_mask
    # overflow rank within (c,w,t)
    cum = np.cumsum(ov_mask)
    gcounts = np.bincount(gs, minlength=cfg.C * W * 2)
    gstarts = np.concatenate([[0], np.cumsum(gcounts)[:-1]])
    cum_at_start = np.where(gstarts > 0, cum[gstarts - 1], 0)
    ovr = cum - 1 - cum_at_start[gs]

    max_ov = int((ovr[ov_mask].max() + 1) if ov_mask.any() else 0)
    if max_ov > OV * 128:
        raise RuntimeError(f"overflow capacity exceeded: {max_ov} > {OV*128}")

    cores_s = (gs // (2 * W)).astype(np.int64)
    w_s = (gs // 2) % W
    ty_s = gs % 2

    # tile index within window and partition
    tile_idx = np.where(id_mask, rank, KI + (ovr >> 7))
    part = np.where(id_mask, js, ovr & 127)
    col = w_s * TT + ty_s * TPT + tile_idx

    ZROW = cfg.N
    off = np.full((C, 128, W * TT), ZROW, np.int32)
    off[cores_s, part, col] = srcs

    sel = np.full((C, 128, W * NOV), -1.0, np.float32)
    ov_idx = np.nonzero(ov_mask)[0]
    scol = (w_s[ov_idx] * NOV + ty_s[ov_idx] * OV
            + (ovr[ov_idx] >> 7))
    sel[cores_s[ov_idx], ovr[ov_idx] & 127, scol] = js[ov_idx].astype(
        np.float32)
    return off, sel


def prep_inputs(cfg: Cfg, x, edge_index, edge_type, w_sl, b_sl,
                w1_n, b1_n, gamma_n, beta_n, w2_n, b2_n,
                w1_d, b1_d, gamma_d, beta_d, w2_d, b2_d,
                w_gat, b_gat):
    C, npc, npad = cfg.C, cfg.npc, cfg.npad
    x = np.asarray(x, np.float32)
    src = np.asarray(edge_index[0], np.int64).astype(np.int64)
    dst = np.asarray(edge_index[1], np.int64).astype(np.int64)
    et = np.asarray(edge_type, np.int64).astype(np.int64)

    off, sel = pack_edges(cfg, src, dst, et)

    xbf = np.vstack([x, np.zeros((1, 128), np.float32)]).astype(
        F8 if MSG_FP8 else BF)
    msgs = [np.ascontiguousarray(xbf[off[c]].reshape(128, -1))
            for c in range(C)]

    xTs = []
    xselfs = []
    W = cfg.W
    for c in range(C):
        xp = np.zeros((npad, 128), np.float32)
        xp[:npc] = x[c * npc:(c + 1) * npc]
        xTs.append(np.ascontiguousarray(xp.T).astype(BF))
        xselfs.append(np.ascontiguousarray(
            xp.reshape(W, 128, 128).transpose(1, 0, 2)
              .reshape(128, npad)).astype(BF))

    def bt(a):
        return np.ascontiguousarray(np.asarray(a, np.float64)).astype(BF)

    w_sl64 = np.asarray(w_sl, np.float64)
    w2n64 = np.asarray(w2_n, np.float64)
    w2d64 = np.asarray(w2_d, np.float64)
    wg = np.asarray(w_gat, np.float64)
    wg0, wg1, wg2 = wg[:, 0:128], wg[:, 128:256], wg[:, 256:384]

    wcols = [
        bt(w_sl64.T), bt(np.asarray(w1_n).T), bt(np.asarray(w1_d).T),
        bt(w2n64.T), bt(w2d64[::-1, :].T),
        bt((wg0 @ w_sl64).T), bt((wg1 @ w2n64).T), bt((wg2 @ w2d64).T),
        bt(np.triu(np.ones((128, 128), np.float32))),
        bt(np.eye(128, dtype=np.float32)),
    ]
    wpack = np.concatenate(wcols, axis=1)

    rows = np.concatenate([
        (np.asarray(b_sl, np.float64) + np.asarray(b2_n, np.float64))[None, :],
        np.asarray(b2_d, np.float64)[::-1][None, :],
    ], axis=1).astype(BF)

    bz = (np.asarray(b_gat, np.float64) + wg0 @ np.asarray(b_sl, np.float64)
          + wg1 @ np.asarray(b2_n, np.float64)
          + wg2 @ np.asarray(b2_d, np.float64))
    vecs = np.stack([
        np.asarray(b1_n, np.float64), np.asarray(b1_d, np.float64),
        np.asarray(gamma_n, np.float64), np.asarray(beta_n, np.float64),
        np.asarray(gamma_d, np.float64), np.asarray(beta_d, np.float64),
        bz,
    ], axis=1).astype(np.float32)

    in_maps = []
    for c in range(C):
        m = {
            "msgs": msgs[c],
            "xT": xTs[c],
            "xself": xselfs[c],
            "sel": np.ascontiguousarray(sel[c]),
            "wpack": wpack,
            "rows": rows,
            "vecs": vecs,
            "iota128": np.broadcast_to(
                np.arange(128, dtype=np.float32)[None, :],
                (128, 128)).copy(),
        }
        if MSG_FP8:
            m["ipair"] = np.concatenate(
                [np.eye(128, dtype=np.float32)] * 2, axis=1).astype(F8)
        in_maps.append(m)
    return in_maps


_BUILD_CACHE = {}


def run(cfg: Cfg, inputs: dict, **run_kwargs):
    in_maps = None
    while True:
        try:
            in_maps = prep_inputs(cfg, **inputs)
            break
        except RuntimeError:
            cfg = Cfg(cfg.N, cfg.E, cfg.C, cfg.KI, cfg.OV + 1)
    key = (cfg.N, cfg.E, cfg.C, cfg.KI, cfg.OV,
           USE_TTR, USE_STT, USE_POOL_HBN, USE_ACT_T1, MSG_FP8)
    if key not in _BUILD_CACHE:
        _BUILD_CACHE[key] = build(cfg)
    nc = _BUILD_CACHE[key]
    res = run_bass_kernel_spmd(nc, in_maps, core_ids=list(range(cfg.C)),
                               **run_kwargs)
    outs = [res.results[c]["out"][:cfg.npc] for c in range(cfg.C)]
    return np.concatenate(outs, axis=0).astype(np.float32), res


def kernel(**inputs):
    out, _ = run(CFG, inputs)
    return out
